# revision 12
# baseline (speedup 1.0000x reference)
"""Bass/Tile HBV kernel for 8 TRN2 NeuronCores.

Bulk reformulation: per chunk of 128 cells (partition dim) x 730 days (free dim),
the HBV recurrences become hardware tensor_tensor_scan instructions plus bulk
elementwise ops; nonlinear buckets are solved by short Picard/Newton iterations
(validated in numpy to converge well below the 2e-2 gate).

v2: engine-balanced instruction stream. The Vector (DVE) engine is the
bottleneck and is SBUF-read-bandwidth bound, so:
 - unary affine ops (scale/bias/relu/exp/ln/copy-cast) run on the Scalar
   engine (activation), including part of the routing-conv tap products;
 - two-ALU-op tensor_scalar fusions replace tensor_tensor pairs wherever a
   per-partition scalar operand allows;
 - the first soil-moisture Newton iteration is linearized around the constant
   FC/2 initial state (host-precomputed coefficients; no Ln/Exp needed);
 - the final soil iteration reuses the previous linearization (chord Newton);
 - the SUZ regime iteration runs in fp16 with a single fp32 polish pass;
 - per-slot iteration counts tuned in a bit-accurate numpy mirror.

Cells are ranked by a host-side difficulty estimate and striped across cores so
each chunk-slot is difficulty-homogeneous; harder slots run more SUZ regime
iterations.

Self-contained: needs numpy + concourse (+ axon TRN2 devices).
"""
import numpy as np

import concourse.bacc as bacc
import concourse.mybir as mybir
from concourse.bass_utils import run_bass_kernel_spmd
from concourse.tile import TileContext

F32 = mybir.dt.float32
F16 = mybir.dt.float16
AL = mybir.AluOpType
AF = mybir.ActivationFunctionType

T = 730
G_FULL = 10000
N_CORES = 8
CHUNKS_PER_CORE = 10
P128 = 128
G_PAD = N_CORES * CHUNKS_PER_CORE * P128  # 10240
LENF = 15
NEARZERO = 1e-5

PHY_BOUNDS = [
    ("parBETA", 1.0, 6.0), ("parFC", 50.0, 1000.0), ("parK0", 0.05, 0.9),
    ("parK1", 0.01, 0.5), ("parK2", 0.001, 0.2), ("parLP", 0.2, 1.0),
    ("parPERC", 0.0, 10.0), ("parUZL", 0.0, 100.0), ("parTT", -2.5, 2.5),
    ("parCFMAX", 0.5, 10.0), ("parCFR", 0.0, 0.1), ("parCWH", 0.0, 0.2),
]
ROUT_A_BOUNDS = (0.0, 2.9)
ROUT_B_BOUNDS = (0.0, 6.5)

# const column indices
(C_TT, C_MS, C_MB, C_RS, C_RB, C_1CWH, C_FC, C_BETA, C_BLIF, C_BM1, C_SWPB,
 C_ILPFC, C_PCAP, C_PCUZ, C_CA, C_CB, C_C3, C_1K2, C_KAP, C_FCH,
 C_USW0, C_NSWP0, C_NPCAP) = range(23)
C_W0 = 23
NCONST = C_W0 + LENF  # 38

# per-slot config; slot 0 = hardest cells (per host difficulty ranking)
SLOT_NS = (13, 7, 7, 5, 4, 3, 3, 3, 3, 3)
SUZ_TAIL = 1          # f32 polish iterations at the end of the SUZ loop
SOIL_PLAN = "chhh"    # c=const-linearized f16, h=f16 Newton, s=f32 Newton,
                      # H=f16 chord (frozen linearization)
N_A = 1               # snow Picard passes
N_B = 4               # soil iterations (= len(SOIL_PLAN); kept for sim.py)
SOIL_SM0 = "equil"    # first-soil-iteration linearization point: equil | half
CONV_TAU = 14         # conv taps whose w*q product runs on the Scalar engine


def _sigmoid(x):
    return 1.0 / (1.0 + np.exp(-x))


def host_params(par_last):
    phy = _sigmoid(par_last[:, :12].astype(np.float64))
    rout = _sigmoid(par_last[:, 12:].astype(np.float64))
    p = {}
    for i, (nm, lo, hi) in enumerate(PHY_BOUNDS):
        p[nm] = lo + phy[:, i] * (hi - lo)
    p["rout_a"] = ROUT_A_BOUNDS[0] + rout[:, 0] * (ROUT_A_BOUNDS[1] - ROUT_A_BOUNDS[0])
    p["rout_b"] = ROUT_B_BOUNDS[0] + rout[:, 1] * (ROUT_B_BOUNDS[1] - ROUT_B_BOUNDS[0])
    return p


def host_consts(p):
    g = len(p["parTT"])
    c = np.zeros((g, NCONST), np.float64)
    TTp = p["parTT"]; CFMAX = p["parCFMAX"]; CFR = p["parCFR"]
    beta = p["parBETA"]; FC = p["parFC"]
    c[:, C_TT] = TTp
    c[:, C_MS] = CFMAX
    c[:, C_MB] = -CFMAX * TTp
    c[:, C_RS] = -CFR * CFMAX
    c[:, C_RB] = CFR * CFMAX * TTp
    c[:, C_1CWH] = 1.0 + p["parCWH"]
    c[:, C_FC] = FC
    c[:, C_BETA] = beta
    lnInvFC = -np.log(FC)
    c[:, C_BLIF] = beta * lnInvFC
    c[:, C_BM1] = beta - 1.0
    c[:, C_SWPB] = beta * lnInvFC + np.log(beta)
    c[:, C_ILPFC] = 1.0 / (p["parLP"] * FC)
    c[:, C_PCAP] = p["parPERC"]
    c[:, C_PCUZ] = p["parPERC"] + p["parUZL"]
    ca = 1.0 - p["parK1"]
    c[:, C_CA] = ca
    c[:, C_CB] = -p["parK0"] * ca
    c[:, C_C3] = ca * p["parK0"] * p["parUZL"]
    c[:, C_1K2] = 1.0 - p["parK2"]
    c[:, C_KAP] = p["parK2"] / (1.0 - p["parK2"])
    # First-soil-iteration linearization/init point SM0: per-cell equilibrium
    # of the soil water balance under mean forcing (host bisection), clamped
    # away from the edges; falls back to FC/2 when no forcing stats given.
    stats = p.get("_forcing_stats")
    if stats is not None and SOIL_SM0 == "equil":
        INm, PETm = stats
        lpfc = p["parLP"] * FC
        lo = np.full_like(FC, 1e-3)
        hi = FC.copy()
        for _ in range(40):
            mid = 0.5 * (lo + hi)
            f = (INm * (1.0 - (mid / FC) ** beta)
                 - PETm * np.minimum(mid / lpfc, 1.0))
            lo = np.where(f > 0, mid, lo)
            hi = np.where(f > 0, hi, mid)
        SM0 = np.clip(0.5 * (lo + hi), 0.02 * FC, 0.98 * FC)
    else:
        SM0 = 0.5 * FC
    c[:, C_FCH] = SM0
    r0 = SM0 / FC
    sw0 = r0 ** beta
    swp0 = (r0 ** (beta - 1.0)) * beta / FC
    c[:, C_USW0] = 1.0 - sw0
    c[:, C_NSWP0] = -swp0
    c[:, C_NPCAP] = -p["parPERC"]
    aa = np.maximum(p["rout_a"], 0.0) + 0.1
    theta = np.maximum(p["rout_b"], 0.0) + 0.5
    tk = np.arange(LENF, dtype=np.float64) + 0.5
    wv = np.exp((aa[:, None] - 1.0) * np.log(tk)[None, :]
                - tk[None, :] / theta[:, None])
    c[:, C_W0:C_W0 + LENF] = wv / wv.sum(axis=1, keepdims=True)
    return c.astype(np.float32)


def difficulty(p, x_phy, stride=4, k_lo=4, k_hi=9):
    """Per-cell SUZ iteration difficulty: residual between k_lo and k_hi regime
    iterations of a coarse (time-strided) SUZ solve with a proxy inflow."""
    P = x_phy[::stride, :, 0].astype(np.float64)
    PET = x_phy[::stride, :, 2].astype(np.float64)
    SUZIN = np.maximum(P - 0.7 * PET, 0.0)
    Tc, G = SUZIN.shape
    K0 = p["parK0"]; K1 = p["parK1"]; PCAP = p["parPERC"]; UZL = p["parUZL"]
    ca = 1.0 - K1
    SUZ_prev = np.zeros((Tc, G))
    keep = {}
    SUZ = np.zeros((Tc, G))
    for it in range(k_hi):
        S1 = SUZ_prev + SUZIN
        m1 = S1 > PCAP
        m2 = S1 > PCAP + UZL
        alpha = ca * (1.0 - K0 * m2) * m1
        beta = alpha * (SUZIN - PCAP) + (ca * K0 * UZL) * m2
        s = np.zeros(G)
        for t in range(Tc):
            s = alpha[t] * s + beta[t]
            SUZ[t] = s
        if it + 1 in (k_lo, k_hi):
            keep[it + 1] = SUZ.copy()
        SUZ_prev[1:] = SUZ[:-1]
        SUZ_prev[0] = 0.0
    return np.abs(keep[k_hi] - keep[k_lo]).mean(axis=0)


def build_nc(slot_ns=SLOT_NS, soil_plan=SOIL_PLAN, n_a=N_A, suz_tail=SUZ_TAIL,
             conv_tau=CONV_TAU):
    nc = bacc.Bacc("TRN2", target_bir_lowering=False, debug=False,
                   num_devices=N_CORES)
    din = {}
    for nm in ("pp", "tm", "pe"):
        din[nm] = nc.declare_dram_parameter(nm, [CHUNKS_PER_CORE, P128, T], F32,
                                            isOutput=False)
    din["cc"] = nc.declare_dram_parameter("cc", [CHUNKS_PER_CORE, P128, NCONST],
                                          F32, isOutput=False)
    dout = nc.declare_dram_parameter("y", [CHUNKS_PER_CORE, P128, T], F32,
                                     isOutput=True)
    with TileContext(nc) as tc:
        with tc.tile_pool(name="gl", bufs=1) as gpool:
            zeros = gpool.tile([P128, T], F32, name="zeros")
            nc.vector.memset(zeros[:, :], 0.0)
            ones = gpool.tile([P128, T], F32, name="ones")
            nc.vector.memset(ones[:, :], 1.0)
            c001 = gpool.tile([P128, 1], F32, name="c001")
            nc.vector.memset(c001[:, :], 0.001)
            with tc.tile_pool(name="io", bufs=2) as iop, \
                    tc.tile_pool(name="wk", bufs=2) as wk:
                for ci in range(CHUNKS_PER_CORE):
                    _chunk(nc, (iop, wk), din, dout, ci, zeros, ones, c001,
                           n_a, soil_plan, slot_ns[ci], suz_tail, conv_tau)
    nc.compile()
    return nc


def _chunk(nc, pools, din, dout, ci, zeros, ones, c001, n_a, plan, n_s, tail,
           tau):
    iop, wk = pools
    V = nc.vector
    A = nc.scalar
    dma = nc.sync.dma_start

    def tl(tag, w=T, dt=F32):
        return wk.tile([P128, w], dt, tag=tag, name=tag)

    # io planes
    Pp = iop.tile([P128, T], F32, tag="Pp", name="Pp")
    TMp = iop.tile([P128, T], F32, tag="TMp", name="TMp")
    PEp = iop.tile([P128, T], F32, tag="PEp", name="PEp")
    ct = iop.tile([P128, NCONST], F32, tag="ct", name="ct")
    dma(Pp[:, :], din["pp"][ci])
    dma(TMp[:, :], din["tm"][ci])
    dma(PEp[:, :], din["pe"][ci])
    dma(ct[:, :], din["cc"][ci])

    def cc(i):
        return ct[:, i:i + 1]

    # scratch planes
    s0 = tl("s0"); s1 = tl("s1"); s2 = tl("s2"); s3 = tl("s3")
    s4 = tl("s4"); s5 = tl("s5"); s6 = tl("s6")
    g0 = tl("g0", T, F16); g1 = tl("g1", T, F16); g2 = tl("g2", T, F16)
    g3 = tl("g3", T, F16); g4 = tl("g4", T, F16); g5 = tl("g5", T, F16)
    g6 = tl("g6", T, F16); g7 = tl("g7", T, F16)
    PEh = tl("PEh", T, F16)
    PETinv16 = tl("PETinv16", T, F16)
    INh = tl("INh", T, F16)
    eSoil = tl("eSoil", T, F16)      # e of the last soil iteration (f16 plans)
    has_H = "H" in plan
    uPrev = tl("uPrev", T, F16) if has_H else g6   # frozen-linearization carry
    aPrev = tl("aPrev", T, F16) if has_H else g5
    ebF32 = tl("ebF32") if "s" in plan else eSoil  # e of an f32 soil iteration

    # ---- stage 0 ----
    SNOW = tl("SNOW"); Aa = tl("Aa")
    PETinv = tl("PETinv") if "s" in plan else None
    negR = tl("negR") if n_a >= 2 else None
    A.activation(s0[:, :], TMp[:, :], AF.Relu, scale=cc(C_MS), bias=cc(C_MB))  # M
    if n_a >= 2:
        A.activation(s1[:, :], TMp[:, :], AF.Relu, scale=cc(C_RS), bias=cc(C_RB))
        A.activation(negR[:, :], s1[:, :], AF.Copy, scale=-1.0)
    V.tensor_scalar(s2[:, :], TMp[:, :], cc(C_TT), None, AL.is_lt)
    V.tensor_tensor(SNOW[:, :], Pp[:, :], s2[:, :], AL.mult)
    V.tensor_tensor(Aa[:, :], SNOW[:, :], s0[:, :], AL.subtract)
    if "s" in plan:
        A.activation(PETinv[:, :], PEp[:, :], AF.Copy, scale=cc(C_ILPFC))
    A.activation(PETinv16[:, :], PEp[:, :], AF.Copy, scale=cc(C_ILPFC))
    A.activation(PEh[:, :], PEp[:, :], AF.Copy)

    # ---- snow ----
    Xb = tl("Xb"); Wb = tl("Wb", T + 1)
    cbuf = tl("cbuf", T + 1) if n_a >= 2 else None
    negMW = tl("negMW", T + 1) if n_a >= 2 else None
    V.memset(Wb[:, 0:1], 0.002)
    if n_a >= 2:
        V.memset(cbuf[:, 0:1], 0.0)
        V.memset(negMW[:, 0:1], -0.001)
    sp = None
    for it in range(n_a):
        if it == 0:
            V.tensor_tensor_scan(Xb[:, :], Aa[:, :], zeros[:, :], 0.001,
                                 AL.add, AL.max)
            sp = Xb
        else:
            V.tensor_tensor(negMW[:, 1:T + 1], sp[:, :], Wb[:, 1:T + 1],
                            AL.subtract)
            V.scalar_tensor_tensor(s0[:, :], negMW[:, 0:T], 0.0, negR[:, :],
                                   AL.min, AL.max)                       # -r
            V.tensor_tensor_scan(cbuf[:, 1:T + 1], s0[:, :], s0[:, :], 0.0,
                                 AL.add, AL.bypass)                      # -cumsum r
            V.tensor_tensor_scan(Xb[:, :], Aa[:, :], cbuf[:, 0:T], 0.001,
                                 AL.add, AL.max)
            V.tensor_tensor(s1[:, :], Xb[:, :], cbuf[:, 1:T + 1], AL.subtract)
            sp = s1
        A.activation(s2[:, :], sp[:, :], AF.Copy, scale=cc(C_1CWH))
        V.tensor_tensor_scan(Wb[:, 1:T + 1], SNOW[:, :], s2[:, :], 0.002,
                             AL.add, AL.min)
    INb = tl("INb")
    V.tensor_tensor(s0[:, :], Wb[:, 0:T], Wb[:, 1:T + 1], AL.subtract)
    V.tensor_tensor(INb[:, :], s0[:, :], Pp[:, :], AL.add)
    A.activation(INh[:, :], INb[:, :], AF.Copy)

    # ---- soil Newton (per-plan-char iterations) ----
    SMb = tl("SMb", T + 1)
    V.memset(SMb[:, 0:1], 0.001)
    A.activation(SMb[:, 1:T + 1], ones[:, :], AF.Copy, scale=cc(C_FCH))
    last_e16 = True
    for it, ch in enumerate(plan):
        if ch == "c":
            # const linearization around SM = FC/2 (host-precomputed coeffs)
            V.tensor_scalar(g1[:, :], INh[:, :], cc(C_USW0), cc(C_FCH),
                            AL.mult, AL.add)                        # SMa
            V.tensor_scalar(g2[:, :], g1[:, :], cc(C_FC), None, AL.min)  # SMmid
            V.tensor_scalar(g5[:, :], g2[:, :], cc(C_ILPFC), 1.0,
                            AL.mult, AL.min)                        # q
            V.tensor_tensor(eSoil[:, :], PEh[:, :], g5[:, :], AL.mult)   # e
            V.tensor_tensor(g0[:, :], g2[:, :], eSoil[:, :], AL.subtract)  # fval
            V.tensor_scalar(g3[:, :], g1[:, :], cc(C_FC), None, AL.is_lt)  # mFC
            V.tensor_scalar(g2[:, :], INh[:, :], cc(C_NSWP0), 1.0,
                            AL.mult, AL.add)                        # 1-IN*swp0
            V.tensor_scalar(g7[:, :], g5[:, :], 1.0, None, AL.is_lt)     # mEF
            V.tensor_tensor(g4[:, :], g7[:, :], PETinv16[:, :], AL.mult)
            A.activation(g7[:, :], g4[:, :], AF.Copy, scale=-1.0, bias=1.0)
            V.tensor_tensor(g4[:, :], g3[:, :], g2[:, :], AL.mult)
            V.tensor_tensor(g3[:, :], g4[:, :], g7[:, :], AL.mult)
            V.tensor_scalar(aPrev[:, :], g3[:, :], 0.0, 1.0, AL.max, AL.min)  # a
            V.tensor_scalar(s0[:, :], g0[:, :], cc(C_FCH), None, AL.subtract)  # rho
            V.tensor_tensor_scan(s1[:, :], aPrev[:, :], s0[:, :], 0.0,
                                 AL.mult, AL.add)
            V.tensor_scalar(s2[:, :], s1[:, :], cc(C_FCH), NEARZERO,
                            AL.add, AL.max)
            V.tensor_scalar(SMb[:, 1:T + 1], s2[:, :], cc(C_FC), None, AL.min)
            if "H" in plan:
                # u carry for a following frozen iteration
                V.tensor_scalar(uPrev[:, :], INh[:, :], cc(C_USW0), None,
                                AL.mult)
            last_e16 = True
        elif ch == "h":
            A.activation(s0[:, :], SMb[:, 0:T], AF.Ln)
            A.activation(g1[:, :], s0[:, :], AF.Exp, scale=cc(C_BETA), bias=cc(C_BLIF))
            A.activation(g2[:, :], s0[:, :], AF.Exp, scale=cc(C_BM1), bias=cc(C_SWPB))
            A.activation(g0[:, :], g1[:, :], AF.Copy, scale=-1.0, bias=1.0)  # 1-sw
            V.tensor_tensor(uPrev[:, :], g0[:, :], INh[:, :], AL.mult)       # u
            V.tensor_tensor(s3[:, :], SMb[:, 0:T], uPrev[:, :], AL.add)      # SMa
            V.tensor_scalar(s4[:, :], s3[:, :], cc(C_FC), None, AL.min)      # SMmid
            V.tensor_scalar(g5[:, :], s4[:, :], cc(C_ILPFC), 1.0, AL.mult, AL.min)  # q
            V.tensor_tensor(eSoil[:, :], PEh[:, :], g5[:, :], AL.mult)       # e
            V.tensor_tensor(s6[:, :], s4[:, :], eSoil[:, :], AL.subtract)    # fval
            V.tensor_scalar(g3[:, :], s3[:, :], cc(C_FC), None, AL.is_lt)    # mFC
            V.tensor_tensor(g4[:, :], INh[:, :], g2[:, :], AL.mult)
            A.activation(g2[:, :], g4[:, :], AF.Copy, scale=-1.0, bias=1.0)  # 1-IN*swp
            V.tensor_scalar(g7[:, :], g5[:, :], 1.0, None, AL.is_lt)         # mEF
            V.tensor_tensor(g4[:, :], g7[:, :], PETinv16[:, :], AL.mult)
            A.activation(g7[:, :], g4[:, :], AF.Copy, scale=-1.0, bias=1.0)
            V.tensor_tensor(g4[:, :], g3[:, :], g2[:, :], AL.mult)
            V.tensor_tensor(g3[:, :], g4[:, :], g7[:, :], AL.mult)
            V.tensor_scalar(aPrev[:, :], g3[:, :], 0.0, 1.0, AL.max, AL.min)  # a
            V.tensor_tensor(s0[:, :], s6[:, :], SMb[:, 1:T + 1], AL.subtract)  # rho
            V.tensor_tensor_scan(s1[:, :], aPrev[:, :], s0[:, :], 0.0,
                                 AL.mult, AL.add)
            V.tensor_tensor(s2[:, :], SMb[:, 1:T + 1], s1[:, :], AL.add)
            V.tensor_scalar(SMb[:, 1:T + 1], s2[:, :], NEARZERO, cc(C_FC),
                            AL.max, AL.min)
            last_e16 = True
        elif ch == "H":
            # chord Newton: reuse uPrev/aPrev, recompute residual only
            V.tensor_tensor(s3[:, :], SMb[:, 0:T], uPrev[:, :], AL.add)      # SMa
            V.tensor_scalar(s4[:, :], s3[:, :], cc(C_FC), None, AL.min)      # SMmid
            V.tensor_scalar(g5[:, :], s4[:, :], cc(C_ILPFC), 1.0, AL.mult, AL.min)  # q
            V.tensor_tensor(eSoil[:, :], PEh[:, :], g5[:, :], AL.mult)       # e
            V.tensor_tensor(s6[:, :], s4[:, :], eSoil[:, :], AL.subtract)    # fval
            V.tensor_tensor(s0[:, :], s6[:, :], SMb[:, 1:T + 1], AL.subtract)  # rho
            V.tensor_tensor_scan(s1[:, :], aPrev[:, :], s0[:, :], 0.0,
                                 AL.mult, AL.add)
            V.tensor_tensor(s2[:, :], SMb[:, 1:T + 1], s1[:, :], AL.add)
            V.tensor_scalar(SMb[:, 1:T + 1], s2[:, :], NEARZERO, cc(C_FC),
                            AL.max, AL.min)
            last_e16 = True
        elif ch == "s":
            A.activation(s0[:, :], SMb[:, 0:T], AF.Ln)
            A.activation(s1[:, :], s0[:, :], AF.Exp, scale=cc(C_BETA), bias=cc(C_BLIF))
            A.activation(s2[:, :], s0[:, :], AF.Exp, scale=cc(C_BM1), bias=cc(C_SWPB))
            A.activation(s0[:, :], s1[:, :], AF.Copy, scale=-1.0, bias=1.0)
            V.tensor_tensor(s1[:, :], s0[:, :], INb[:, :], AL.mult)          # u
            V.tensor_tensor(s3[:, :], SMb[:, 0:T], s1[:, :], AL.add)         # SMa
            V.tensor_scalar(s4[:, :], s3[:, :], cc(C_FC), None, AL.min)      # SMmid
            V.tensor_scalar(s5[:, :], s4[:, :], cc(C_ILPFC), 1.0, AL.mult, AL.min)
            V.tensor_tensor(ebF32[:, :], PEp[:, :], s5[:, :], AL.mult)       # e
            V.tensor_tensor(s6[:, :], s4[:, :], ebF32[:, :], AL.subtract)    # fval
            V.tensor_scalar(s0[:, :], s3[:, :], cc(C_FC), None, AL.is_lt)
            V.tensor_tensor(s1[:, :], INb[:, :], s2[:, :], AL.mult)
            A.activation(s1[:, :], s1[:, :], AF.Copy, scale=-1.0, bias=1.0)
            V.tensor_scalar(s2[:, :], s5[:, :], 1.0, None, AL.is_lt)
            V.tensor_tensor(s2[:, :], s2[:, :], PETinv[:, :], AL.mult)
            A.activation(s2[:, :], s2[:, :], AF.Copy, scale=-1.0, bias=1.0)
            V.tensor_tensor(s3[:, :], s0[:, :], s1[:, :], AL.mult)
            V.tensor_tensor(s4[:, :], s3[:, :], s2[:, :], AL.mult)
            V.tensor_scalar(s5[:, :], s4[:, :], 0.0, 1.0, AL.max, AL.min)
            V.tensor_tensor(s0[:, :], s6[:, :], SMb[:, 1:T + 1], AL.subtract)
            V.tensor_tensor_scan(s1[:, :], s5[:, :], s0[:, :], 0.0,
                                 AL.mult, AL.add)
            V.tensor_tensor(s2[:, :], SMb[:, 1:T + 1], s1[:, :], AL.add)
            V.tensor_scalar(SMb[:, 1:T + 1], s2[:, :], NEARZERO, cc(C_FC),
                            AL.max, AL.min)
            last_e16 = False
        else:
            raise ValueError(f"bad soil plan char {ch}")

    # SUZIN = (INb - e) - (SM[1:] - SM[:T])
    SUZIN = tl("SUZIN")
    esrc = eSoil if last_e16 else ebF32
    V.tensor_tensor(s0[:, :], INb[:, :], esrc[:, :], AL.subtract)
    V.tensor_tensor(s1[:, :], SMb[:, 1:T + 1], SMb[:, 0:T], AL.subtract)
    V.tensor_tensor(SUZIN[:, :], s0[:, :], s1[:, :], AL.subtract)

    # ---- SUZ regime iteration: f16 with f32 polish tail ----
    SUZb = tl("SUZb", T + 1); SINP = tl("SINP")
    SUZh = tl("SUZh", T + 1, F16)
    SUZINh = tl("SUZINh", T, F16); SINPh = tl("SINPh", T, F16)
    h0 = tl("h0", T, F16); h2 = tl("h2", T, F16); h3 = tl("h3", T, F16)
    h4 = tl("h4", T, F16); h5 = tl("h5", T, F16); h6 = tl("h6", T, F16)
    V.memset(SUZb[:, 0:1], 0.001)
    V.memset(SUZh[:, 0:1], 0.001)
    A.activation(SINP[:, :], SUZIN[:, :], AF.Identity, bias=cc(C_NPCAP))
    A.activation(SUZINh[:, :], SUZIN[:, :], AF.Copy)
    A.activation(SINPh[:, :], SINP[:, :], AF.Copy)
    for it in range(n_s):
        if it < n_s - tail:
            if it == 0:
                A.activation(h0[:, :], SUZINh[:, :], AF.Identity,
                             bias=c001[:, 0:1])
            else:
                V.tensor_tensor(h0[:, :], SUZh[:, 0:T], SUZINh[:, :], AL.add)
            V.tensor_scalar(h3[:, :], h0[:, :], cc(C_PCAP), cc(C_CA),
                            AL.is_gt, AL.mult)
            V.tensor_scalar(h6[:, :], h0[:, :], cc(C_PCUZ), cc(C_CB),
                            AL.is_gt, AL.mult)
            V.tensor_tensor(h4[:, :], h3[:, :], h6[:, :], AL.add)      # alpha
            V.tensor_tensor(h5[:, :], h4[:, :], SINPh[:, :], AL.mult)
            V.tensor_scalar(h2[:, :], h0[:, :], cc(C_PCUZ), cc(C_C3),
                            AL.is_gt, AL.mult)
            V.tensor_tensor(h3[:, :], h5[:, :], h2[:, :], AL.add)      # beta
            V.tensor_tensor_scan(SUZh[:, 1:T + 1], h4[:, :], h3[:, :], 0.001,
                                 AL.mult, AL.add)
            continue
        prevb = SUZh if it == n_s - tail else SUZb
        V.tensor_tensor(s0[:, :], prevb[:, 0:T], SUZIN[:, :], AL.add)  # S1
        V.tensor_scalar(s3[:, :], s0[:, :], cc(C_PCAP), cc(C_CA),
                        AL.is_gt, AL.mult)
        V.tensor_scalar(s4[:, :], s0[:, :], cc(C_PCUZ), cc(C_CB),
                        AL.is_gt, AL.mult)
        V.tensor_tensor(s5[:, :], s3[:, :], s4[:, :], AL.add)          # alpha
        V.tensor_tensor(s6[:, :], s5[:, :], SINP[:, :], AL.mult)
        V.tensor_scalar(s2[:, :], s0[:, :], cc(C_PCUZ), cc(C_C3),
                        AL.is_gt, AL.mult)
        V.tensor_tensor(s3[:, :], s6[:, :], s2[:, :], AL.add)          # beta
        V.tensor_tensor_scan(SUZb[:, 1:T + 1], s5[:, :], s3[:, :], 0.001,
                             AL.mult, AL.add)

    # ---- post-SUZ / SLZ ----
    V.tensor_tensor(s0[:, :], SUZb[:, 0:T], SUZIN[:, :], AL.add)       # S1
    V.tensor_scalar(s5[:, :], s0[:, :], cc(C_PCAP), cc(C_1K2),
                    AL.min, AL.mult)                                   # (1-K2)*PERC
    A.activation(s2[:, :], s0[:, :], AF.Relu, bias=cc(C_NPCAP))        # S1-PERC
    V.tensor_tensor(s3[:, :], s2[:, :], SUZb[:, 1:T + 1], AL.subtract)  # Q01
    A.activation(s4[:, :], ones[:, :], AF.Copy, scale=cc(C_1K2))
    V.tensor_tensor_scan(s6[:, :], s4[:, :], s5[:, :], 0.001, AL.mult, AL.add)  # SLZ
    A.activation(s0[:, :], s6[:, :], AF.Copy, scale=cc(C_KAP))         # Q2

    # ---- routing conv in fp16 (tap products split V/Scalar) ----
    QbH = tl("QbH", T + LENF - 1, F16)
    yA, yB, wq0, wq1 = g0, g1, g2, g3
    P = nc.gpsimd
    V.memset(QbH[:, 0:LENF - 1], 0.0)
    V.tensor_tensor(QbH[:, LENF - 1:T + LENF - 1], s3[:, :], s0[:, :], AL.add)  # Q
    base = LENF - 1
    A.activation(yA[:, :], QbH[:, base:base + T], AF.Copy, scale=cc(C_W0))
    src, dst = yA, yB
    for k in range(1, LENF):
        wq = (wq0, wq1)[k % 2]
        if k <= tau:
            A.activation(wq[:, :], QbH[:, base - k:base - k + T], AF.Copy,
                         scale=cc(C_W0 + k))
        else:
            V.tensor_scalar(wq[:, :], QbH[:, base - k:base - k + T],
                            cc(C_W0 + k), None, AL.mult)
        # accumulate on the (otherwise idle) Pool engine; chunk i's chain
        # overlaps chunk i+1's Vector work, and the output DMA is a Pool
        # instruction anyway
        P.tensor_tensor(dst[:, :], src[:, :], wq[:, :], AL.add)
        src, dst = dst, src
    nc.gpsimd.dma_start(dout[ci], src[:, :])  # casts fp16 -> fp32


# ---------------- host orchestration ----------------
_CACHE = {}


def _get_nc(key=None):
    if key is None:
        key = (SLOT_NS, SOIL_PLAN, N_A, SUZ_TAIL, CONV_TAU)
    if key not in _CACHE:
        _CACHE[key] = build_nc(*key)
    return _CACHE[key]


def cell_layout(p, x_phy):
    """Position i (0..G_PAD-1) holds source cell cells[i]; chunk g=i//128 maps
    to core g%8, slot g//8. Hardest cells first so low slots are hard.
    Difficulty = union of two rankings: coarse-sim SUZ residual and the
    persistence heuristic (a cell is hard if either says so)."""
    G = G_FULL
    dsim = difficulty(p, x_phy)
    Pm = x_phy[:, :, 0].mean(axis=0).astype(np.float64)
    PETm = x_phy[:, :, 2].mean(axis=0).astype(np.float64)
    dcrude = (1.0 - p["parK1"]) * (Pm - 0.7 * PETm > p["parPERC"])

    def ranks(d):
        o = np.argsort(-d, kind="stable")
        r = np.empty(G, np.int64)
        r[o] = np.arange(G)
        return r

    runion = np.minimum(ranks(dsim), ranks(dcrude))
    pad = np.arange(G_PAD - G)
    rall = np.concatenate([runion, runion[pad]])
    order = np.argsort(rall, kind="stable")
    cells = np.concatenate([np.arange(G), pad])[order]
    gchunk = np.arange(G_PAD) // P128
    core_of = gchunk % N_CORES
    return cells, core_of


def kernel(x_phy: np.ndarray, parameters: np.ndarray, trace=False):
    x = np.asarray(x_phy, np.float32)
    par_last = np.asarray(parameters)[-1].astype(np.float32)
    Tn, G, _ = x.shape
    assert Tn == T and G == G_FULL
    p = host_params(par_last)
    cells, core_of = cell_layout(p, x)
    p["_forcing_stats"] = (x[:, :, 0].mean(axis=0).astype(np.float64),
                           x[:, :, 2].mean(axis=0).astype(np.float64))
    consts_all = host_consts(p)[cells]
    xg = x[:, cells, :]
    in_maps = []
    per_core = CHUNKS_PER_CORE * P128
    for c in range(N_CORES):
        idx = np.where(core_of == c)[0]
        blk = np.ascontiguousarray(np.moveaxis(xg[:, idx, :], 0, 1))  # [pc, T, 3]
        in_maps.append({
            "pp": np.ascontiguousarray(blk[:, :, 0]).reshape(CHUNKS_PER_CORE, P128, T),
            "tm": np.ascontiguousarray(blk[:, :, 1]).reshape(CHUNKS_PER_CORE, P128, T),
            "pe": np.ascontiguousarray(blk[:, :, 2]).reshape(CHUNKS_PER_CORE, P128, T),
            "cc": np.ascontiguousarray(consts_all[idx]).reshape(CHUNKS_PER_CORE, P128, NCONST),
        })
    nc = _get_nc()
    res = run_bass_kernel_spmd(nc, in_maps, list(range(N_CORES)), trace=trace)
    out = np.empty((T, G), np.float32)
    for c in range(N_CORES):
        idx = np.where(core_of == c)[0]
        ys = res.results[c]["y"].reshape(per_core, T)
        out[:, cells[idx]] = ys.T  # pad duplicates overwrite identically
    if trace:
        return out, res
    return out


# revision 14
# speedup vs baseline: 1.0267x; 1.0267x over previous
"""Bass/Tile HBV kernel for 8 TRN2 NeuronCores.

Bulk reformulation: per chunk of 128 cells (partition dim) x 730 days (free dim),
the HBV recurrences become hardware tensor_tensor_scan instructions plus bulk
elementwise ops; nonlinear buckets are solved by short Picard/Newton iterations
(validated in numpy to converge well below the 2e-2 gate).

v2: engine-balanced instruction stream. The Vector (DVE) engine is the
bottleneck and is SBUF-read-bandwidth bound, so:
 - unary affine ops (scale/bias/relu/exp/ln/copy-cast) run on the Scalar
   engine (activation), including part of the routing-conv tap products;
 - two-ALU-op tensor_scalar fusions replace tensor_tensor pairs wherever a
   per-partition scalar operand allows;
 - the first soil-moisture Newton iteration is linearized around the constant
   FC/2 initial state (host-precomputed coefficients; no Ln/Exp needed);
 - the final soil iteration reuses the previous linearization (chord Newton);
 - the SUZ regime iteration runs in fp16 with a single fp32 polish pass;
 - per-slot iteration counts tuned in a bit-accurate numpy mirror.

Cells are ranked by a host-side difficulty estimate and striped across cores so
each chunk-slot is difficulty-homogeneous; harder slots run more SUZ regime
iterations.

Self-contained: needs numpy + concourse (+ axon TRN2 devices).
"""
import numpy as np

import concourse.bacc as bacc
import concourse.mybir as mybir
from concourse.bass_utils import run_bass_kernel_spmd
from concourse.tile import TileContext

F32 = mybir.dt.float32
F16 = mybir.dt.float16
AL = mybir.AluOpType
AF = mybir.ActivationFunctionType

T = 730
G_FULL = 10000
N_CORES = 8
CHUNKS_PER_CORE = 10
P128 = 128
G_PAD = N_CORES * CHUNKS_PER_CORE * P128  # 10240
LENF = 15
NEARZERO = 1e-5

PHY_BOUNDS = [
    ("parBETA", 1.0, 6.0), ("parFC", 50.0, 1000.0), ("parK0", 0.05, 0.9),
    ("parK1", 0.01, 0.5), ("parK2", 0.001, 0.2), ("parLP", 0.2, 1.0),
    ("parPERC", 0.0, 10.0), ("parUZL", 0.0, 100.0), ("parTT", -2.5, 2.5),
    ("parCFMAX", 0.5, 10.0), ("parCFR", 0.0, 0.1), ("parCWH", 0.0, 0.2),
]
ROUT_A_BOUNDS = (0.0, 2.9)
ROUT_B_BOUNDS = (0.0, 6.5)

# const column indices
(C_TT, C_MS, C_MB, C_RS, C_RB, C_1CWH, C_FC, C_BETA, C_BLIF, C_BM1, C_SWPB,
 C_ILPFC, C_PCAP, C_PCUZ, C_CA, C_CB, C_C3, C_1K2, C_KAP, C_FCH,
 C_USW0, C_NSWP0, C_NPCAP) = range(23)
C_W0 = 23
NCONST = C_W0 + LENF  # 38

# per-slot config; slot 0 = hardest cells (per host difficulty ranking)
SLOT_NS = (12, 6, 6, 4, 4, 3, 3, 3, 3, 3)
SUZ_TAIL = 1          # f32 polish iterations at the end of the SUZ loop
SOIL_PLAN = "chhh"    # c=const-linearized f16, h=f16 Newton, s=f32 Newton,
                      # H=f16 chord (frozen linearization)
N_A = 1               # snow Picard passes
N_B = 4               # soil iterations (= len(SOIL_PLAN); kept for sim.py)
SOIL_SM0 = "half"     # first-soil-iteration linearization point: equil | half
CONV_TAU = 0          # unused (conv runs on the PE via diagonal matmuls)


def _sigmoid(x):
    return 1.0 / (1.0 + np.exp(-x))


def host_params(par_last):
    phy = _sigmoid(par_last[:, :12].astype(np.float64))
    rout = _sigmoid(par_last[:, 12:].astype(np.float64))
    p = {}
    for i, (nm, lo, hi) in enumerate(PHY_BOUNDS):
        p[nm] = lo + phy[:, i] * (hi - lo)
    p["rout_a"] = ROUT_A_BOUNDS[0] + rout[:, 0] * (ROUT_A_BOUNDS[1] - ROUT_A_BOUNDS[0])
    p["rout_b"] = ROUT_B_BOUNDS[0] + rout[:, 1] * (ROUT_B_BOUNDS[1] - ROUT_B_BOUNDS[0])
    return p


def host_consts(p):
    g = len(p["parTT"])
    c = np.zeros((g, NCONST), np.float64)
    TTp = p["parTT"]; CFMAX = p["parCFMAX"]; CFR = p["parCFR"]
    beta = p["parBETA"]; FC = p["parFC"]
    c[:, C_TT] = TTp
    c[:, C_MS] = CFMAX
    c[:, C_MB] = -CFMAX * TTp
    c[:, C_RS] = -CFR * CFMAX
    c[:, C_RB] = CFR * CFMAX * TTp
    c[:, C_1CWH] = 1.0 + p["parCWH"]
    c[:, C_FC] = FC
    c[:, C_BETA] = beta
    lnInvFC = -np.log(FC)
    c[:, C_BLIF] = beta * lnInvFC
    c[:, C_BM1] = beta - 1.0
    c[:, C_SWPB] = beta * lnInvFC + np.log(beta)
    c[:, C_ILPFC] = 1.0 / (p["parLP"] * FC)
    c[:, C_PCAP] = p["parPERC"]
    c[:, C_PCUZ] = p["parPERC"] + p["parUZL"]
    ca = 1.0 - p["parK1"]
    c[:, C_CA] = ca
    c[:, C_CB] = -p["parK0"] * ca
    c[:, C_C3] = ca * p["parK0"] * p["parUZL"]
    c[:, C_1K2] = 1.0 - p["parK2"]
    c[:, C_KAP] = p["parK2"] / (1.0 - p["parK2"])
    # First-soil-iteration linearization/init point SM0: per-cell equilibrium
    # of the soil water balance under mean forcing (host bisection), clamped
    # away from the edges; falls back to FC/2 when no forcing stats given.
    stats = p.get("_forcing_stats")
    if stats is not None and SOIL_SM0 == "equil":
        INm, PETm = stats
        lpfc = p["parLP"] * FC
        lo = np.full_like(FC, 1e-3)
        hi = FC.copy()
        for _ in range(40):
            mid = 0.5 * (lo + hi)
            f = (INm * (1.0 - (mid / FC) ** beta)
                 - PETm * np.minimum(mid / lpfc, 1.0))
            lo = np.where(f > 0, mid, lo)
            hi = np.where(f > 0, hi, mid)
        SM0 = np.clip(0.5 * (lo + hi), 0.02 * FC, 0.98 * FC)
    else:
        SM0 = 0.5 * FC
    c[:, C_FCH] = SM0
    r0 = SM0 / FC
    sw0 = r0 ** beta
    swp0 = (r0 ** (beta - 1.0)) * beta / FC
    c[:, C_USW0] = 1.0 - sw0
    c[:, C_NSWP0] = -swp0
    c[:, C_NPCAP] = -p["parPERC"]
    aa = np.maximum(p["rout_a"], 0.0) + 0.1
    theta = np.maximum(p["rout_b"], 0.0) + 0.5
    tk = np.arange(LENF, dtype=np.float64) + 0.5
    wv = np.exp((aa[:, None] - 1.0) * np.log(tk)[None, :]
                - tk[None, :] / theta[:, None])
    c[:, C_W0:C_W0 + LENF] = wv / wv.sum(axis=1, keepdims=True)
    return c.astype(np.float32)


def difficulty(p, x_phy, stride=4, k_lo=4, k_hi=9):
    """Per-cell SUZ iteration difficulty: residual between k_lo and k_hi regime
    iterations of a coarse (time-strided) SUZ solve with a proxy inflow."""
    P = x_phy[::stride, :, 0].astype(np.float64)
    PET = x_phy[::stride, :, 2].astype(np.float64)
    SUZIN = np.maximum(P - 0.7 * PET, 0.0)
    Tc, G = SUZIN.shape
    K0 = p["parK0"]; K1 = p["parK1"]; PCAP = p["parPERC"]; UZL = p["parUZL"]
    ca = 1.0 - K1
    SUZ_prev = np.zeros((Tc, G))
    keep = {}
    SUZ = np.zeros((Tc, G))
    for it in range(k_hi):
        S1 = SUZ_prev + SUZIN
        m1 = S1 > PCAP
        m2 = S1 > PCAP + UZL
        alpha = ca * (1.0 - K0 * m2) * m1
        beta = alpha * (SUZIN - PCAP) + (ca * K0 * UZL) * m2
        s = np.zeros(G)
        for t in range(Tc):
            s = alpha[t] * s + beta[t]
            SUZ[t] = s
        if it + 1 in (k_lo, k_hi):
            keep[it + 1] = SUZ.copy()
        SUZ_prev[1:] = SUZ[:-1]
        SUZ_prev[0] = 0.0
    return np.abs(keep[k_hi] - keep[k_lo]).mean(axis=0)


def build_nc(slot_ns=SLOT_NS, soil_plan=SOIL_PLAN, n_a=N_A, suz_tail=SUZ_TAIL,
             conv_tau=CONV_TAU):
    nc = bacc.Bacc("TRN2", target_bir_lowering=False, debug=False,
                   num_devices=N_CORES)
    din = {}
    for nm in ("pp", "tm", "pe"):
        din[nm] = nc.declare_dram_parameter(nm, [CHUNKS_PER_CORE, P128, T], F32,
                                            isOutput=False)
    din["cc"] = nc.declare_dram_parameter("cc", [CHUNKS_PER_CORE, P128, NCONST],
                                          F32, isOutput=False)
    din["wd"] = nc.declare_dram_parameter("wd", [CHUNKS_PER_CORE, P128,
                                                 LENF * P128], F16,
                                          isOutput=False)
    dout = nc.declare_dram_parameter("y", [CHUNKS_PER_CORE, P128, T], F32,
                                     isOutput=True)
    with TileContext(nc) as tc:
        with tc.tile_pool(name="gl", bufs=1) as gpool:
            zeros = gpool.tile([P128, T], F32, name="zeros")
            nc.vector.memset(zeros[:, :], 0.0)
            ones = gpool.tile([P128, T], F32, name="ones")
            nc.vector.memset(ones[:, :], 1.0)
            c001 = gpool.tile([P128, 1], F32, name="c001")
            nc.vector.memset(c001[:, :], 0.001)
            with tc.tile_pool(name="io", bufs=2) as iop, \
                    tc.tile_pool(name="wk", bufs=2) as wk, \
                    tc.psum_pool(name="ps", bufs=2) as psp:
                for ci in range(CHUNKS_PER_CORE):
                    _chunk(nc, (iop, wk, psp), din, dout, ci, zeros, ones,
                           c001, n_a, soil_plan, slot_ns[ci], suz_tail,
                           conv_tau)
    nc.compile()
    return nc


def _chunk(nc, pools, din, dout, ci, zeros, ones, c001, n_a, plan, n_s, tail,
           tau):
    iop, wk, psp = pools
    V = nc.vector
    A = nc.scalar
    dma = nc.sync.dma_start

    def tl(tag, w=T, dt=F32):
        return wk.tile([P128, w], dt, tag=tag, name=tag)

    # io planes
    Pp = iop.tile([P128, T], F32, tag="Pp", name="Pp")
    TMp = iop.tile([P128, T], F32, tag="TMp", name="TMp")
    PEp = iop.tile([P128, T], F32, tag="PEp", name="PEp")
    ct = iop.tile([P128, NCONST], F32, tag="ct", name="ct")
    wdt = iop.tile([P128, LENF * P128], F16, tag="wdt", name="wdt")
    dma(Pp[:, :], din["pp"][ci])
    dma(TMp[:, :], din["tm"][ci])
    dma(PEp[:, :], din["pe"][ci])
    dma(ct[:, :], din["cc"][ci])
    dma(wdt[:, :], din["wd"][ci])

    def cc(i):
        return ct[:, i:i + 1]

    # scratch planes
    s0 = tl("s0"); s1 = tl("s1"); s2 = tl("s2"); s3 = tl("s3")
    s4 = tl("s4"); s5 = tl("s5"); s6 = tl("s6")
    g0 = tl("g0", T, F16); g1 = tl("g1", T, F16); g2 = tl("g2", T, F16)
    g3 = tl("g3", T, F16); g4 = tl("g4", T, F16); g5 = tl("g5", T, F16)
    g6 = tl("g6", T, F16); g7 = tl("g7", T, F16)
    PEh = tl("PEh", T, F16)
    PETinv16 = tl("PETinv16", T, F16)
    INh = tl("INh", T, F16)
    eSoil = tl("eSoil", T, F16)      # e of the last soil iteration (f16 plans)
    has_H = "H" in plan
    has_P = "P" in plan
    uPrev = tl("uPrev", T, F16) if has_H else g6   # frozen-linearization carry
    aPrev = tl("aPrev", T, F16) if (has_H or has_P) else g5
    ebF32 = tl("ebF32") if "s" in plan else eSoil  # e of an f32 soil iteration

    # ---- stage 0 ----
    SNOW = tl("SNOW"); Aa = tl("Aa")
    PETinv = tl("PETinv") if "s" in plan else None
    negR = tl("negR") if n_a >= 2 else None
    A.activation(s0[:, :], TMp[:, :], AF.Relu, scale=cc(C_MS), bias=cc(C_MB))  # M
    if n_a >= 2:
        A.activation(s1[:, :], TMp[:, :], AF.Relu, scale=cc(C_RS), bias=cc(C_RB))
        A.activation(negR[:, :], s1[:, :], AF.Copy, scale=-1.0)
    V.tensor_scalar(s2[:, :], TMp[:, :], cc(C_TT), None, AL.is_lt)
    V.tensor_tensor(SNOW[:, :], Pp[:, :], s2[:, :], AL.mult)
    V.tensor_tensor(Aa[:, :], SNOW[:, :], s0[:, :], AL.subtract)
    if "s" in plan:
        A.activation(PETinv[:, :], PEp[:, :], AF.Copy, scale=cc(C_ILPFC))
    A.activation(PETinv16[:, :], PEp[:, :], AF.Copy, scale=cc(C_ILPFC))
    A.activation(PEh[:, :], PEp[:, :], AF.Copy)

    # ---- snow ----
    Xb = tl("Xb"); Wb = tl("Wb", T + 1)
    cbuf = tl("cbuf", T + 1) if n_a >= 2 else None
    negMW = tl("negMW", T + 1) if n_a >= 2 else None
    V.memset(Wb[:, 0:1], 0.002)
    if n_a >= 2:
        V.memset(cbuf[:, 0:1], 0.0)
        V.memset(negMW[:, 0:1], -0.001)
    sp = None
    for it in range(n_a):
        if it == 0:
            V.tensor_tensor_scan(Xb[:, :], Aa[:, :], zeros[:, :], 0.001,
                                 AL.add, AL.max)
            sp = Xb
        else:
            V.tensor_tensor(negMW[:, 1:T + 1], sp[:, :], Wb[:, 1:T + 1],
                            AL.subtract)
            V.scalar_tensor_tensor(s0[:, :], negMW[:, 0:T], 0.0, negR[:, :],
                                   AL.min, AL.max)                       # -r
            V.tensor_tensor_scan(cbuf[:, 1:T + 1], s0[:, :], s0[:, :], 0.0,
                                 AL.add, AL.bypass)                      # -cumsum r
            V.tensor_tensor_scan(Xb[:, :], Aa[:, :], cbuf[:, 0:T], 0.001,
                                 AL.add, AL.max)
            V.tensor_tensor(s1[:, :], Xb[:, :], cbuf[:, 1:T + 1], AL.subtract)
            sp = s1
        A.activation(s2[:, :], sp[:, :], AF.Copy, scale=cc(C_1CWH))
        V.tensor_tensor_scan(Wb[:, 1:T + 1], SNOW[:, :], s2[:, :], 0.002,
                             AL.add, AL.min)
    INb = tl("INb")
    V.tensor_tensor(s0[:, :], Wb[:, 0:T], Wb[:, 1:T + 1], AL.subtract)
    V.tensor_tensor(INb[:, :], s0[:, :], Pp[:, :], AL.add)
    A.activation(INh[:, :], INb[:, :], AF.Copy)

    # ---- soil Newton (per-plan-char iterations) ----
    SMb = tl("SMb", T + 1)
    V.memset(SMb[:, 0:1], 0.001)
    A.activation(SMb[:, 1:T + 1], ones[:, :], AF.Copy, scale=cc(C_FCH))
    last_e16 = True
    for it, ch in enumerate(plan):
        if ch == "c":
            # const linearization around SM = FC/2 (host-precomputed coeffs)
            V.tensor_scalar(g1[:, :], INh[:, :], cc(C_USW0), cc(C_FCH),
                            AL.mult, AL.add)                        # SMa
            V.tensor_scalar(g2[:, :], g1[:, :], cc(C_FC), None, AL.min)  # SMmid
            V.tensor_scalar(g5[:, :], g2[:, :], cc(C_ILPFC), 1.0,
                            AL.mult, AL.min)                        # q
            V.tensor_tensor(eSoil[:, :], PEh[:, :], g5[:, :], AL.mult)   # e
            V.tensor_tensor(g0[:, :], g2[:, :], eSoil[:, :], AL.subtract)  # fval
            V.tensor_scalar(g3[:, :], g1[:, :], cc(C_FC), None, AL.is_lt)  # mFC
            V.tensor_scalar(g2[:, :], INh[:, :], cc(C_NSWP0), 1.0,
                            AL.mult, AL.add)                        # 1-IN*swp0
            V.tensor_scalar(g7[:, :], g5[:, :], 1.0, None, AL.is_lt)     # mEF
            V.tensor_tensor(g4[:, :], g7[:, :], PETinv16[:, :], AL.mult)
            A.activation(g7[:, :], g4[:, :], AF.Copy, scale=-1.0, bias=1.0)
            V.tensor_tensor(g4[:, :], g3[:, :], g2[:, :], AL.mult)
            V.tensor_tensor(g3[:, :], g4[:, :], g7[:, :], AL.mult)
            V.tensor_scalar(aPrev[:, :], g3[:, :], 0.0, 1.0, AL.max, AL.min)  # a
            V.tensor_scalar(s0[:, :], g0[:, :], cc(C_FCH), None, AL.subtract)  # rho
            V.tensor_tensor_scan(s1[:, :], aPrev[:, :], s0[:, :], 0.0,
                                 AL.mult, AL.add)
            V.tensor_scalar(s2[:, :], s1[:, :], cc(C_FCH), NEARZERO,
                            AL.add, AL.max)
            V.tensor_scalar(SMb[:, 1:T + 1], s2[:, :], cc(C_FC), None, AL.min)
            if "H" in plan:
                # u carry for a following frozen iteration
                V.tensor_scalar(uPrev[:, :], INh[:, :], cc(C_USW0), None,
                                AL.mult)
            last_e16 = True
        elif ch == "h":
            A.activation(s0[:, :], SMb[:, 0:T], AF.Ln)
            A.activation(g1[:, :], s0[:, :], AF.Exp, scale=cc(C_BETA), bias=cc(C_BLIF))
            A.activation(g2[:, :], s0[:, :], AF.Exp, scale=cc(C_BM1), bias=cc(C_SWPB))
            A.activation(g0[:, :], g1[:, :], AF.Copy, scale=-1.0, bias=1.0)  # 1-sw
            V.tensor_tensor(uPrev[:, :], g0[:, :], INh[:, :], AL.mult)       # u
            V.tensor_tensor(s3[:, :], SMb[:, 0:T], uPrev[:, :], AL.add)      # SMa
            V.tensor_scalar(s4[:, :], s3[:, :], cc(C_FC), None, AL.min)      # SMmid
            V.tensor_scalar(g5[:, :], s4[:, :], cc(C_ILPFC), 1.0, AL.mult, AL.min)  # q
            V.tensor_tensor(eSoil[:, :], PEh[:, :], g5[:, :], AL.mult)       # e
            V.tensor_tensor(s6[:, :], s4[:, :], eSoil[:, :], AL.subtract)    # fval
            V.tensor_scalar(g3[:, :], s3[:, :], cc(C_FC), None, AL.is_lt)    # mFC
            V.tensor_tensor(g4[:, :], INh[:, :], g2[:, :], AL.mult)
            A.activation(g2[:, :], g4[:, :], AF.Copy, scale=-1.0, bias=1.0)  # 1-IN*swp
            V.tensor_scalar(g7[:, :], g5[:, :], 1.0, None, AL.is_lt)         # mEF
            V.tensor_tensor(g4[:, :], g7[:, :], PETinv16[:, :], AL.mult)
            A.activation(g7[:, :], g4[:, :], AF.Copy, scale=-1.0, bias=1.0)
            V.tensor_tensor(g4[:, :], g3[:, :], g2[:, :], AL.mult)
            V.tensor_tensor(g3[:, :], g4[:, :], g7[:, :], AL.mult)
            V.tensor_scalar(aPrev[:, :], g3[:, :], 0.0, 1.0, AL.max, AL.min)  # a
            V.tensor_tensor(s0[:, :], s6[:, :], SMb[:, 1:T + 1], AL.subtract)  # rho
            V.tensor_tensor_scan(s1[:, :], aPrev[:, :], s0[:, :], 0.0,
                                 AL.mult, AL.add)
            V.tensor_tensor(s2[:, :], SMb[:, 1:T + 1], s1[:, :], AL.add)
            V.tensor_scalar(SMb[:, 1:T + 1], s2[:, :], NEARZERO, cc(C_FC),
                            AL.max, AL.min)
            last_e16 = True
        elif ch == "H":
            # chord Newton: reuse uPrev/aPrev, recompute residual only
            V.tensor_tensor(s3[:, :], SMb[:, 0:T], uPrev[:, :], AL.add)      # SMa
            V.tensor_scalar(s4[:, :], s3[:, :], cc(C_FC), None, AL.min)      # SMmid
            V.tensor_scalar(g5[:, :], s4[:, :], cc(C_ILPFC), 1.0, AL.mult, AL.min)  # q
            V.tensor_tensor(eSoil[:, :], PEh[:, :], g5[:, :], AL.mult)       # e
            V.tensor_tensor(s6[:, :], s4[:, :], eSoil[:, :], AL.subtract)    # fval
            V.tensor_tensor(s0[:, :], s6[:, :], SMb[:, 1:T + 1], AL.subtract)  # rho
            V.tensor_tensor_scan(s1[:, :], aPrev[:, :], s0[:, :], 0.0,
                                 AL.mult, AL.add)
            V.tensor_tensor(s2[:, :], SMb[:, 1:T + 1], s1[:, :], AL.add)
            V.tensor_scalar(SMb[:, 1:T + 1], s2[:, :], NEARZERO, cc(C_FC),
                            AL.max, AL.min)
            last_e16 = True
        elif ch == "P":
            A.activation(s0[:, :], SMb[:, 0:T], AF.Ln)
            A.activation(g1[:, :], s0[:, :], AF.Exp, scale=cc(C_BETA), bias=cc(C_BLIF))
            A.activation(g0[:, :], g1[:, :], AF.Copy, scale=-1.0, bias=1.0)  # 1-sw
            V.tensor_tensor(g6[:, :], g0[:, :], INh[:, :], AL.mult)          # u
            V.tensor_tensor(s3[:, :], SMb[:, 0:T], g6[:, :], AL.add)         # SMa
            V.tensor_scalar(s4[:, :], s3[:, :], cc(C_FC), None, AL.min)      # SMmid
            V.tensor_scalar(g5[:, :], s4[:, :], cc(C_ILPFC), 1.0, AL.mult, AL.min)  # q
            V.tensor_tensor(eSoil[:, :], PEh[:, :], g5[:, :], AL.mult)       # e
            V.tensor_tensor(s6[:, :], s4[:, :], eSoil[:, :], AL.subtract)    # fval
            V.tensor_tensor(s0[:, :], s6[:, :], SMb[:, 1:T + 1], AL.subtract)  # rho
            V.tensor_tensor_scan(s1[:, :], aPrev[:, :], s0[:, :], 0.0,
                                 AL.mult, AL.add)
            V.tensor_tensor(s2[:, :], SMb[:, 1:T + 1], s1[:, :], AL.add)
            V.tensor_scalar(SMb[:, 1:T + 1], s2[:, :], NEARZERO, cc(C_FC),
                            AL.max, AL.min)
            last_e16 = True
        elif ch == "s":
            A.activation(s0[:, :], SMb[:, 0:T], AF.Ln)
            A.activation(s1[:, :], s0[:, :], AF.Exp, scale=cc(C_BETA), bias=cc(C_BLIF))
            A.activation(s2[:, :], s0[:, :], AF.Exp, scale=cc(C_BM1), bias=cc(C_SWPB))
            A.activation(s0[:, :], s1[:, :], AF.Copy, scale=-1.0, bias=1.0)
            V.tensor_tensor(s1[:, :], s0[:, :], INb[:, :], AL.mult)          # u
            V.tensor_tensor(s3[:, :], SMb[:, 0:T], s1[:, :], AL.add)         # SMa
            V.tensor_scalar(s4[:, :], s3[:, :], cc(C_FC), None, AL.min)      # SMmid
            V.tensor_scalar(s5[:, :], s4[:, :], cc(C_ILPFC), 1.0, AL.mult, AL.min)
            V.tensor_tensor(ebF32[:, :], PEp[:, :], s5[:, :], AL.mult)       # e
            V.tensor_tensor(s6[:, :], s4[:, :], ebF32[:, :], AL.subtract)    # fval
            V.tensor_scalar(s0[:, :], s3[:, :], cc(C_FC), None, AL.is_lt)
            V.tensor_tensor(s1[:, :], INb[:, :], s2[:, :], AL.mult)
            A.activation(s1[:, :], s1[:, :], AF.Copy, scale=-1.0, bias=1.0)
            V.tensor_scalar(s2[:, :], s5[:, :], 1.0, None, AL.is_lt)
            V.tensor_tensor(s2[:, :], s2[:, :], PETinv[:, :], AL.mult)
            A.activation(s2[:, :], s2[:, :], AF.Copy, scale=-1.0, bias=1.0)
            V.tensor_tensor(s3[:, :], s0[:, :], s1[:, :], AL.mult)
            V.tensor_tensor(s4[:, :], s3[:, :], s2[:, :], AL.mult)
            V.tensor_scalar(s5[:, :], s4[:, :], 0.0, 1.0, AL.max, AL.min)
            V.tensor_tensor(s0[:, :], s6[:, :], SMb[:, 1:T + 1], AL.subtract)
            V.tensor_tensor_scan(s1[:, :], s5[:, :], s0[:, :], 0.0,
                                 AL.mult, AL.add)
            V.tensor_tensor(s2[:, :], SMb[:, 1:T + 1], s1[:, :], AL.add)
            V.tensor_scalar(SMb[:, 1:T + 1], s2[:, :], NEARZERO, cc(C_FC),
                            AL.max, AL.min)
            last_e16 = False
        else:
            raise ValueError(f"bad soil plan char {ch}")

    # SUZIN = (INb - e) - (SM[1:] - SM[:T])
    SUZIN = tl("SUZIN")
    esrc = eSoil if last_e16 else ebF32
    V.tensor_tensor(s0[:, :], INb[:, :], esrc[:, :], AL.subtract)
    V.tensor_tensor(s1[:, :], SMb[:, 1:T + 1], SMb[:, 0:T], AL.subtract)
    V.tensor_tensor(SUZIN[:, :], s0[:, :], s1[:, :], AL.subtract)

    # ---- SUZ regime iteration: f16 with f32 polish tail ----
    SUZb = tl("SUZb", T + 1); SINP = tl("SINP")
    SUZh = tl("SUZh", T + 1, F16)
    SUZINh = tl("SUZINh", T, F16); SINPh = tl("SINPh", T, F16)
    h0 = tl("h0", T, F16); h2 = tl("h2", T, F16); h3 = tl("h3", T, F16)
    h4 = tl("h4", T, F16); h5 = tl("h5", T, F16); h6 = tl("h6", T, F16)
    V.memset(SUZb[:, 0:1], 0.001)
    V.memset(SUZh[:, 0:1], 0.001)
    A.activation(SINP[:, :], SUZIN[:, :], AF.Identity, bias=cc(C_NPCAP))
    A.activation(SUZINh[:, :], SUZIN[:, :], AF.Copy)
    A.activation(SINPh[:, :], SINP[:, :], AF.Copy)
    for it in range(n_s):
        if it < n_s - tail:
            if it == 0:
                A.activation(h0[:, :], SUZINh[:, :], AF.Identity,
                             bias=c001[:, 0:1])
            else:
                V.tensor_tensor(h0[:, :], SUZh[:, 0:T], SUZINh[:, :], AL.add)
            V.tensor_scalar(h3[:, :], h0[:, :], cc(C_PCAP), cc(C_CA),
                            AL.is_gt, AL.mult)
            V.tensor_scalar(h6[:, :], h0[:, :], cc(C_PCUZ), cc(C_CB),
                            AL.is_gt, AL.mult)
            V.tensor_tensor(h4[:, :], h3[:, :], h6[:, :], AL.add)      # alpha
            V.tensor_tensor(h5[:, :], h4[:, :], SINPh[:, :], AL.mult)
            V.tensor_scalar(h2[:, :], h0[:, :], cc(C_PCUZ), cc(C_C3),
                            AL.is_gt, AL.mult)
            V.tensor_tensor(h3[:, :], h5[:, :], h2[:, :], AL.add)      # beta
            V.tensor_tensor_scan(SUZh[:, 1:T + 1], h4[:, :], h3[:, :], 0.001,
                                 AL.mult, AL.add)
            continue
        prevb = SUZh if it == n_s - tail else SUZb
        V.tensor_tensor(s0[:, :], prevb[:, 0:T], SUZIN[:, :], AL.add)  # S1
        V.tensor_scalar(s3[:, :], s0[:, :], cc(C_PCAP), cc(C_CA),
                        AL.is_gt, AL.mult)
        V.tensor_scalar(s4[:, :], s0[:, :], cc(C_PCUZ), cc(C_CB),
                        AL.is_gt, AL.mult)
        V.tensor_tensor(s5[:, :], s3[:, :], s4[:, :], AL.add)          # alpha
        V.tensor_tensor(s6[:, :], s5[:, :], SINP[:, :], AL.mult)
        V.tensor_scalar(s2[:, :], s0[:, :], cc(C_PCUZ), cc(C_C3),
                        AL.is_gt, AL.mult)
        V.tensor_tensor(s3[:, :], s6[:, :], s2[:, :], AL.add)          # beta
        V.tensor_tensor_scan(SUZb[:, 1:T + 1], s5[:, :], s3[:, :], 0.001,
                             AL.mult, AL.add)

    # ---- post-SUZ / SLZ ----
    V.tensor_tensor(s0[:, :], SUZb[:, 0:T], SUZIN[:, :], AL.add)       # S1
    V.tensor_scalar(s5[:, :], s0[:, :], cc(C_PCAP), cc(C_1K2),
                    AL.min, AL.mult)                                   # (1-K2)*PERC
    A.activation(s2[:, :], s0[:, :], AF.Relu, bias=cc(C_NPCAP))        # S1-PERC
    V.tensor_tensor(s3[:, :], s2[:, :], SUZb[:, 1:T + 1], AL.subtract)  # Q01
    A.activation(s4[:, :], ones[:, :], AF.Copy, scale=cc(C_1K2))
    V.tensor_tensor_scan(s6[:, :], s4[:, :], s5[:, :], 0.001, AL.mult, AL.add)  # SLZ
    A.activation(s0[:, :], s6[:, :], AF.Copy, scale=cc(C_KAP))         # Q2

    # ---- routing conv on the Tensor engine ----
    # Each tap is a per-cell scalar multiply of a time-shifted Q = a diagonal
    # [128,128] matmul; the 15 taps accumulate exactly in fp32 PSUM. Weights
    # arrive as host-built diagonal matrices (wdt). The moving free dim is
    # capped at 512, so T=730 splits into two PSUM banks of 365 columns.
    QbH = tl("QbH", T + LENF - 1, F16)
    V.memset(QbH[:, 0:LENF - 1], 0.0)
    V.tensor_tensor(QbH[:, LENF - 1:T + LENF - 1], s3[:, :], s0[:, :], AL.add)  # Q
    base = LENF - 1
    TH = T // 2  # 365
    ps0 = psp.tile([P128, TH], F32, tag="ps0", name="ps0")
    ps1 = psp.tile([P128, T - TH], F32, tag="ps1", name="ps1")
    for k in range(LENF):
        wdk = wdt[:, k * P128:(k + 1) * P128]
        nc.tensor.matmul(ps0[:, :], wdk, QbH[:, base - k:base - k + TH],
                         start=(k == 0), stop=(k == LENF - 1))
        nc.tensor.matmul(ps1[:, :], wdk, QbH[:, base - k + TH:base - k + T],
                         start=(k == 0), stop=(k == LENF - 1))
    A.activation(s2[:, 0:TH], ps0[:, :], AF.Copy)
    A.activation(s2[:, TH:T], ps1[:, :], AF.Copy)
    nc.gpsimd.dma_start(dout[ci], s2[:, :])


# ---------------- host orchestration ----------------
_CACHE = {}


def _get_nc(key=None):
    if key is None:
        key = (SLOT_NS, SOIL_PLAN, N_A, SUZ_TAIL, CONV_TAU)
    if key not in _CACHE:
        _CACHE[key] = build_nc(*key)
    return _CACHE[key]


def cell_layout(p, x_phy):
    """Position i (0..G_PAD-1) holds source cell cells[i]; chunk g=i//128 maps
    to core g%8, slot g//8. Hardest cells first so low slots are hard.
    Difficulty = union of two rankings: coarse-sim SUZ residual and the
    persistence heuristic (a cell is hard if either says so)."""
    G = G_FULL
    dsim = difficulty(p, x_phy)
    Pm = x_phy[:, :, 0].mean(axis=0).astype(np.float64)
    PETm = x_phy[:, :, 2].mean(axis=0).astype(np.float64)
    dcrude = (1.0 - p["parK1"]) * (Pm - 0.7 * PETm > p["parPERC"])

    def ranks(d):
        o = np.argsort(-d, kind="stable")
        r = np.empty(G, np.int64)
        r[o] = np.arange(G)
        return r

    runion = np.minimum(ranks(dsim), ranks(dcrude))
    pad = np.arange(G_PAD - G)
    rall = np.concatenate([runion, runion[pad]])
    order = np.argsort(rall, kind="stable")
    cells = np.concatenate([np.arange(G), pad])[order]
    gchunk = np.arange(G_PAD) // P128
    core_of = gchunk % N_CORES
    return cells, core_of


def kernel(x_phy: np.ndarray, parameters: np.ndarray, trace=False):
    x = np.asarray(x_phy, np.float32)
    par_last = np.asarray(parameters)[-1].astype(np.float32)
    Tn, G, _ = x.shape
    assert Tn == T and G == G_FULL
    p = host_params(par_last)
    cells, core_of = cell_layout(p, x)
    p["_forcing_stats"] = (x[:, :, 0].mean(axis=0).astype(np.float64),
                           x[:, :, 2].mean(axis=0).astype(np.float64))
    consts_all = host_consts(p)[cells]
    xg = x[:, cells, :]
    in_maps = []
    per_core = CHUNKS_PER_CORE * P128
    for c in range(N_CORES):
        idx = np.where(core_of == c)[0]
        blk = np.ascontiguousarray(np.moveaxis(xg[:, idx, :], 0, 1))  # [pc, T, 3]
        wch = consts_all[idx][:, C_W0:C_W0 + LENF].astype(np.float16)
        wch = wch.reshape(CHUNKS_PER_CORE, P128, LENF)
        wd = np.zeros((CHUNKS_PER_CORE, P128, LENF * P128), np.float16)
        prng = np.arange(P128)
        for k in range(LENF):
            wd[:, prng, k * P128 + prng] = wch[:, :, k]
        in_maps.append({
            "pp": np.ascontiguousarray(blk[:, :, 0]).reshape(CHUNKS_PER_CORE, P128, T),
            "tm": np.ascontiguousarray(blk[:, :, 1]).reshape(CHUNKS_PER_CORE, P128, T),
            "pe": np.ascontiguousarray(blk[:, :, 2]).reshape(CHUNKS_PER_CORE, P128, T),
            "cc": np.ascontiguousarray(consts_all[idx]).reshape(CHUNKS_PER_CORE, P128, NCONST),
            "wd": wd,
        })
    nc = _get_nc()
    res = run_bass_kernel_spmd(nc, in_maps, list(range(N_CORES)), trace=trace)
    out = np.empty((T, G), np.float32)
    for c in range(N_CORES):
        idx = np.where(core_of == c)[0]
        ys = res.results[c]["y"].reshape(per_core, T)
        out[:, cells[idx]] = ys.T  # pad duplicates overwrite identically
    if trace:
        return out, res
    return out


# revision 15
# speedup vs baseline: 1.1404x; 1.1108x over previous
"""Bass/Tile HBV kernel for 8 TRN2 NeuronCores.

Bulk reformulation: per chunk of 128 cells (partition dim) x 730 days (free dim),
the HBV recurrences become hardware tensor_tensor_scan instructions plus bulk
elementwise ops; nonlinear buckets are solved by short Picard/Newton iterations
(validated in numpy to converge well below the 2e-2 gate).

v2: engine-balanced instruction stream. The Vector (DVE) engine is the
bottleneck and is SBUF-read-bandwidth bound, so:
 - unary affine ops (scale/bias/relu/exp/ln/copy-cast) run on the Scalar
   engine (activation), including part of the routing-conv tap products;
 - two-ALU-op tensor_scalar fusions replace tensor_tensor pairs wherever a
   per-partition scalar operand allows;
 - the first soil-moisture Newton iteration is linearized around the constant
   FC/2 initial state (host-precomputed coefficients; no Ln/Exp needed);
 - the final soil iteration reuses the previous linearization (chord Newton);
 - the SUZ regime iteration runs in fp16 with a single fp32 polish pass;
 - per-slot iteration counts tuned in a bit-accurate numpy mirror.

Cells are ranked by a host-side difficulty estimate and striped across cores so
each chunk-slot is difficulty-homogeneous; harder slots run more SUZ regime
iterations.

Self-contained: needs numpy + concourse (+ axon TRN2 devices).
"""
import numpy as np

import concourse.bacc as bacc
import concourse.mybir as mybir
from concourse.bass_utils import run_bass_kernel_spmd
from concourse.tile import TileContext

F32 = mybir.dt.float32
F16 = mybir.dt.float16
AL = mybir.AluOpType
AF = mybir.ActivationFunctionType

T = 730
G_FULL = 10000
N_CORES = 8
CHUNKS_PER_CORE = 10
P128 = 128
G_PAD = N_CORES * CHUNKS_PER_CORE * P128  # 10240
LENF = 15
NEARZERO = 1e-5

PHY_BOUNDS = [
    ("parBETA", 1.0, 6.0), ("parFC", 50.0, 1000.0), ("parK0", 0.05, 0.9),
    ("parK1", 0.01, 0.5), ("parK2", 0.001, 0.2), ("parLP", 0.2, 1.0),
    ("parPERC", 0.0, 10.0), ("parUZL", 0.0, 100.0), ("parTT", -2.5, 2.5),
    ("parCFMAX", 0.5, 10.0), ("parCFR", 0.0, 0.1), ("parCWH", 0.0, 0.2),
]
ROUT_A_BOUNDS = (0.0, 2.9)
ROUT_B_BOUNDS = (0.0, 6.5)

# const column indices
(C_TT, C_MS, C_MB, C_RS, C_RB, C_1CWH, C_FC, C_BETA, C_BLIF, C_BM1, C_SWPB,
 C_ILPFC, C_PCAP, C_PCUZ, C_CA, C_CB, C_C3, C_1K2, C_KAP, C_FCH,
 C_USW0, C_NSWP0, C_NPCAP, C_NFCH) = range(24)
C_W0 = 24
NCONST = C_W0 + LENF  # 39

# per-slot config; slot 0 = hardest cells (per host difficulty ranking)
SLOT_NS = (12, 6, 6, 4, 4, 3, 3, 3, 3, 3)
SUZ_TAIL = 1          # f32 polish iterations at the end of the SUZ loop
SOIL_PLAN = "chhh"    # c=const-linearized f16, h=f16 Newton, s=f32 Newton,
                      # H=f16 chord (frozen linearization)
N_A = 1               # snow Picard passes
N_B = 4               # soil iterations (= len(SOIL_PLAN); kept for sim.py)
SOIL_SM0 = "half"     # first-soil-iteration linearization point: equil | half
CONV_TAU = 10         # conv taps whose w*q product runs on the Scalar engine


def _sigmoid(x):
    return 1.0 / (1.0 + np.exp(-x))


def host_params(par_last):
    phy = _sigmoid(par_last[:, :12].astype(np.float64))
    rout = _sigmoid(par_last[:, 12:].astype(np.float64))
    p = {}
    for i, (nm, lo, hi) in enumerate(PHY_BOUNDS):
        p[nm] = lo + phy[:, i] * (hi - lo)
    p["rout_a"] = ROUT_A_BOUNDS[0] + rout[:, 0] * (ROUT_A_BOUNDS[1] - ROUT_A_BOUNDS[0])
    p["rout_b"] = ROUT_B_BOUNDS[0] + rout[:, 1] * (ROUT_B_BOUNDS[1] - ROUT_B_BOUNDS[0])
    return p


def host_consts(p):
    g = len(p["parTT"])
    c = np.zeros((g, NCONST), np.float64)
    TTp = p["parTT"]; CFMAX = p["parCFMAX"]; CFR = p["parCFR"]
    beta = p["parBETA"]; FC = p["parFC"]
    c[:, C_TT] = TTp
    c[:, C_MS] = CFMAX
    c[:, C_MB] = -CFMAX * TTp
    c[:, C_RS] = -CFR * CFMAX
    c[:, C_RB] = CFR * CFMAX * TTp
    c[:, C_1CWH] = 1.0 + p["parCWH"]
    c[:, C_FC] = FC
    c[:, C_BETA] = beta
    lnInvFC = -np.log(FC)
    c[:, C_BLIF] = beta * lnInvFC
    c[:, C_BM1] = beta - 1.0
    c[:, C_SWPB] = beta * lnInvFC + np.log(beta)
    c[:, C_ILPFC] = 1.0 / (p["parLP"] * FC)
    c[:, C_PCAP] = p["parPERC"]
    c[:, C_PCUZ] = p["parPERC"] + p["parUZL"]
    ca = 1.0 - p["parK1"]
    c[:, C_CA] = ca
    c[:, C_CB] = -p["parK0"] * ca
    c[:, C_C3] = ca * p["parK0"] * p["parUZL"]
    c[:, C_1K2] = 1.0 - p["parK2"]
    c[:, C_KAP] = p["parK2"] / (1.0 - p["parK2"])
    # First-soil-iteration linearization/init point SM0: per-cell equilibrium
    # of the soil water balance under mean forcing (host bisection), clamped
    # away from the edges; falls back to FC/2 when no forcing stats given.
    stats = p.get("_forcing_stats")
    if stats is not None and SOIL_SM0 == "equil":
        INm, PETm = stats
        lpfc = p["parLP"] * FC
        lo = np.full_like(FC, 1e-3)
        hi = FC.copy()
        for _ in range(40):
            mid = 0.5 * (lo + hi)
            f = (INm * (1.0 - (mid / FC) ** beta)
                 - PETm * np.minimum(mid / lpfc, 1.0))
            lo = np.where(f > 0, mid, lo)
            hi = np.where(f > 0, hi, mid)
        SM0 = np.clip(0.5 * (lo + hi), 0.02 * FC, 0.98 * FC)
    else:
        SM0 = 0.5 * FC
    c[:, C_FCH] = SM0
    r0 = SM0 / FC
    sw0 = r0 ** beta
    swp0 = (r0 ** (beta - 1.0)) * beta / FC
    c[:, C_USW0] = 1.0 - sw0
    c[:, C_NSWP0] = -swp0
    c[:, C_NPCAP] = -p["parPERC"]
    c[:, C_NFCH] = -c[:, C_FCH]
    aa = np.maximum(p["rout_a"], 0.0) + 0.1
    theta = np.maximum(p["rout_b"], 0.0) + 0.5
    tk = np.arange(LENF, dtype=np.float64) + 0.5
    wv = np.exp((aa[:, None] - 1.0) * np.log(tk)[None, :]
                - tk[None, :] / theta[:, None])
    c[:, C_W0:C_W0 + LENF] = wv / wv.sum(axis=1, keepdims=True)
    return c.astype(np.float32)


def difficulty(p, x_phy, stride=4, k_lo=4, k_hi=9):
    """Per-cell SUZ iteration difficulty: residual between k_lo and k_hi regime
    iterations of a coarse (time-strided) SUZ solve with a proxy inflow."""
    P = x_phy[::stride, :, 0].astype(np.float64)
    PET = x_phy[::stride, :, 2].astype(np.float64)
    SUZIN = np.maximum(P - 0.7 * PET, 0.0)
    Tc, G = SUZIN.shape
    K0 = p["parK0"]; K1 = p["parK1"]; PCAP = p["parPERC"]; UZL = p["parUZL"]
    ca = 1.0 - K1
    SUZ_prev = np.zeros((Tc, G))
    keep = {}
    SUZ = np.zeros((Tc, G))
    for it in range(k_hi):
        S1 = SUZ_prev + SUZIN
        m1 = S1 > PCAP
        m2 = S1 > PCAP + UZL
        alpha = ca * (1.0 - K0 * m2) * m1
        beta = alpha * (SUZIN - PCAP) + (ca * K0 * UZL) * m2
        s = np.zeros(G)
        for t in range(Tc):
            s = alpha[t] * s + beta[t]
            SUZ[t] = s
        if it + 1 in (k_lo, k_hi):
            keep[it + 1] = SUZ.copy()
        SUZ_prev[1:] = SUZ[:-1]
        SUZ_prev[0] = 0.0
    return np.abs(keep[k_hi] - keep[k_lo]).mean(axis=0)


def build_nc(slot_ns=SLOT_NS, soil_plan=SOIL_PLAN, n_a=N_A, suz_tail=SUZ_TAIL,
             conv_tau=CONV_TAU):
    nc = bacc.Bacc("TRN2", target_bir_lowering=False, debug=False,
                   num_devices=N_CORES)
    din = {}
    for nm in ("pp", "tm", "pe"):
        din[nm] = nc.declare_dram_parameter(nm, [CHUNKS_PER_CORE, P128, T], F32,
                                            isOutput=False)
    din["cc"] = nc.declare_dram_parameter("cc", [CHUNKS_PER_CORE, P128, NCONST],
                                          F32, isOutput=False)
    dout = nc.declare_dram_parameter("y", [CHUNKS_PER_CORE, P128, T], F32,
                                     isOutput=True)
    with TileContext(nc) as tc:
        with tc.tile_pool(name="gl", bufs=1) as gpool:
            zeros = gpool.tile([P128, T], F32, name="zeros")
            nc.vector.memset(zeros[:, :], 0.0)
            ones = gpool.tile([P128, T], F32, name="ones")
            nc.vector.memset(ones[:, :], 1.0)
            c001 = gpool.tile([P128, 1], F32, name="c001")
            nc.vector.memset(c001[:, :], 0.001)
            with tc.tile_pool(name="io", bufs=2) as iop, \
                    tc.tile_pool(name="wk", bufs=2) as wk:
                for ci in range(CHUNKS_PER_CORE):
                    _chunk(nc, (iop, wk), din, dout, ci, zeros, ones,
                           c001, n_a, soil_plan, slot_ns[ci], suz_tail,
                           conv_tau)
    nc.compile()
    return nc


def _chunk(nc, pools, din, dout, ci, zeros, ones, c001, n_a, plan, n_s, tail,
           tau):
    iop, wk = pools
    V = nc.vector
    A = nc.scalar
    dma = nc.sync.dma_start

    def tl(tag, w=T, dt=F32):
        return wk.tile([P128, w], dt, tag=tag, name=tag)

    # io planes
    Pp = iop.tile([P128, T], F32, tag="Pp", name="Pp")
    TMp = iop.tile([P128, T], F32, tag="TMp", name="TMp")
    PEp = iop.tile([P128, T], F32, tag="PEp", name="PEp")
    ct = iop.tile([P128, NCONST], F32, tag="ct", name="ct")
    dma(Pp[:, :], din["pp"][ci])
    dma(TMp[:, :], din["tm"][ci])
    dma(PEp[:, :], din["pe"][ci])
    dma(ct[:, :], din["cc"][ci])

    def cc(i):
        return ct[:, i:i + 1]

    # scratch planes
    s0 = tl("s0"); s1 = tl("s1"); s2 = tl("s2"); s3 = tl("s3")
    s4 = tl("s4"); s5 = tl("s5"); s6 = tl("s6")
    g0 = tl("g0", T, F16); g1 = tl("g1", T, F16); g2 = tl("g2", T, F16)
    g3 = tl("g3", T, F16); g4 = tl("g4", T, F16); g5 = tl("g5", T, F16)
    g6 = tl("g6", T, F16); g7 = tl("g7", T, F16)
    PEh = tl("PEh", T, F16)
    PETinv16 = tl("PETinv16", T, F16)
    INh = tl("INh", T, F16)
    eSoil = tl("eSoil", T, F16)      # e of the last soil iteration (f16 plans)
    has_H = "H" in plan
    has_P = "P" in plan
    uPrev = tl("uPrev", T, F16) if has_H else g6   # frozen-linearization carry
    aPrev = tl("aPrev", T, F16) if (has_H or has_P) else g5
    ebF32 = tl("ebF32") if "s" in plan else eSoil  # e of an f32 soil iteration

    # ---- stage 0 ----
    SNOW = tl("SNOW"); Aa = tl("Aa")
    PETinv = tl("PETinv") if "s" in plan else None
    negR = tl("negR") if n_a >= 2 else None
    A.activation(s0[:, :], TMp[:, :], AF.Relu, scale=cc(C_MS), bias=cc(C_MB))  # M
    if n_a >= 2:
        A.activation(s1[:, :], TMp[:, :], AF.Relu, scale=cc(C_RS), bias=cc(C_RB))
        A.activation(negR[:, :], s1[:, :], AF.Copy, scale=-1.0)
    V.tensor_scalar(s2[:, :], TMp[:, :], cc(C_TT), None, AL.is_lt)
    V.tensor_tensor(SNOW[:, :], Pp[:, :], s2[:, :], AL.mult)
    V.tensor_tensor(Aa[:, :], SNOW[:, :], s0[:, :], AL.subtract)
    if "s" in plan:
        A.activation(PETinv[:, :], PEp[:, :], AF.Copy, scale=cc(C_ILPFC))
    A.activation(PETinv16[:, :], PEp[:, :], AF.Copy, scale=cc(C_ILPFC))
    A.activation(PEh[:, :], PEp[:, :], AF.Copy)

    # ---- snow ----
    Xb = tl("Xb"); Wb = tl("Wb", T + 1)
    cbuf = tl("cbuf", T + 1) if n_a >= 2 else None
    negMW = tl("negMW", T + 1) if n_a >= 2 else None
    V.memset(Wb[:, 0:1], 0.002)
    if n_a >= 2:
        V.memset(cbuf[:, 0:1], 0.0)
        V.memset(negMW[:, 0:1], -0.001)
    sp = None
    for it in range(n_a):
        if it == 0:
            V.tensor_tensor_scan(Xb[:, :], Aa[:, :], zeros[:, :], 0.001,
                                 AL.add, AL.max)
            sp = Xb
        else:
            V.tensor_tensor(negMW[:, 1:T + 1], sp[:, :], Wb[:, 1:T + 1],
                            AL.subtract)
            V.scalar_tensor_tensor(s0[:, :], negMW[:, 0:T], 0.0, negR[:, :],
                                   AL.min, AL.max)                       # -r
            V.tensor_tensor_scan(cbuf[:, 1:T + 1], s0[:, :], s0[:, :], 0.0,
                                 AL.add, AL.bypass)                      # -cumsum r
            V.tensor_tensor_scan(Xb[:, :], Aa[:, :], cbuf[:, 0:T], 0.001,
                                 AL.add, AL.max)
            V.tensor_tensor(s1[:, :], Xb[:, :], cbuf[:, 1:T + 1], AL.subtract)
            sp = s1
        A.activation(s2[:, :], sp[:, :], AF.Copy, scale=cc(C_1CWH))
        V.tensor_tensor_scan(Wb[:, 1:T + 1], SNOW[:, :], s2[:, :], 0.002,
                             AL.add, AL.min)
    INb = tl("INb")
    V.tensor_tensor(s0[:, :], Wb[:, 0:T], Wb[:, 1:T + 1], AL.subtract)
    V.tensor_tensor(INb[:, :], s0[:, :], Pp[:, :], AL.add)
    A.activation(INh[:, :], INb[:, :], AF.Copy)

    # ---- soil Newton (per-plan-char iterations) ----
    SMb = tl("SMb", T + 1)
    V.memset(SMb[:, 0:1], 0.001)
    A.activation(SMb[:, 1:T + 1], ones[:, :], AF.Copy, scale=cc(C_FCH))
    last_e16 = True
    for it, ch in enumerate(plan):
        if ch == "c":
            # const linearization around SM = FC/2 (host-precomputed coeffs)
            A.activation(g1[:, :], INh[:, :], AF.Identity, scale=cc(C_USW0),
                         bias=cc(C_FCH))                            # SMa
            V.tensor_scalar(g2[:, :], g1[:, :], cc(C_FC), None, AL.min)  # SMmid
            V.tensor_scalar(g5[:, :], g2[:, :], cc(C_ILPFC), 1.0,
                            AL.mult, AL.min)                        # q
            V.tensor_tensor(eSoil[:, :], PEh[:, :], g5[:, :], AL.mult)   # e
            V.tensor_tensor(g0[:, :], g2[:, :], eSoil[:, :], AL.subtract)  # fval
            V.tensor_scalar(g3[:, :], g1[:, :], cc(C_FC), None, AL.is_lt)  # mFC
            A.activation(g2[:, :], INh[:, :], AF.Identity, scale=cc(C_NSWP0),
                         bias=1.0)                                  # 1-IN*swp0
            V.tensor_scalar(g7[:, :], g5[:, :], 1.0, None, AL.is_lt)     # mEF
            V.tensor_tensor(g4[:, :], g7[:, :], PETinv16[:, :], AL.mult)
            A.activation(g7[:, :], g4[:, :], AF.Copy, scale=-1.0, bias=1.0)
            V.tensor_tensor(g4[:, :], g3[:, :], g2[:, :], AL.mult)
            V.tensor_tensor(g3[:, :], g4[:, :], g7[:, :], AL.mult)
            V.tensor_scalar(aPrev[:, :], g3[:, :], 0.0, 1.0, AL.max, AL.min)  # a
            A.activation(s0[:, :], g0[:, :], AF.Identity, scale=1.0,
                         bias=cc(C_NFCH))                           # rho
            V.tensor_tensor_scan(s1[:, :], aPrev[:, :], s0[:, :], 0.0,
                                 AL.mult, AL.add)
            V.tensor_scalar(s2[:, :], s1[:, :], cc(C_FCH), NEARZERO,
                            AL.add, AL.max)
            V.tensor_scalar(SMb[:, 1:T + 1], s2[:, :], cc(C_FC), None, AL.min)
            if "H" in plan:
                # u carry for a following frozen iteration
                V.tensor_scalar(uPrev[:, :], INh[:, :], cc(C_USW0), None,
                                AL.mult)
            last_e16 = True
        elif ch == "h":
            A.activation(s0[:, :], SMb[:, 0:T], AF.Ln)
            A.activation(g1[:, :], s0[:, :], AF.Exp, scale=cc(C_BETA), bias=cc(C_BLIF))
            A.activation(g2[:, :], s0[:, :], AF.Exp, scale=cc(C_BM1), bias=cc(C_SWPB))
            A.activation(g0[:, :], g1[:, :], AF.Copy, scale=-1.0, bias=1.0)  # 1-sw
            V.tensor_tensor(uPrev[:, :], g0[:, :], INh[:, :], AL.mult)       # u
            V.tensor_tensor(s3[:, :], SMb[:, 0:T], uPrev[:, :], AL.add)      # SMa
            V.tensor_scalar(s4[:, :], s3[:, :], cc(C_FC), None, AL.min)      # SMmid
            V.tensor_scalar(g5[:, :], s4[:, :], cc(C_ILPFC), 1.0, AL.mult, AL.min)  # q
            V.tensor_tensor(eSoil[:, :], PEh[:, :], g5[:, :], AL.mult)       # e
            V.tensor_tensor(s6[:, :], s4[:, :], eSoil[:, :], AL.subtract)    # fval
            V.tensor_scalar(g3[:, :], s3[:, :], cc(C_FC), None, AL.is_lt)    # mFC
            V.tensor_tensor(g4[:, :], INh[:, :], g2[:, :], AL.mult)
            A.activation(g2[:, :], g4[:, :], AF.Copy, scale=-1.0, bias=1.0)  # 1-IN*swp
            V.tensor_scalar(g7[:, :], g5[:, :], 1.0, None, AL.is_lt)         # mEF
            V.tensor_tensor(g4[:, :], g7[:, :], PETinv16[:, :], AL.mult)
            A.activation(g7[:, :], g4[:, :], AF.Copy, scale=-1.0, bias=1.0)
            V.tensor_tensor(g4[:, :], g3[:, :], g2[:, :], AL.mult)
            V.tensor_tensor(g3[:, :], g4[:, :], g7[:, :], AL.mult)
            V.tensor_scalar(aPrev[:, :], g3[:, :], 0.0, 1.0, AL.max, AL.min)  # a
            V.tensor_tensor(s0[:, :], s6[:, :], SMb[:, 1:T + 1], AL.subtract)  # rho
            V.tensor_tensor_scan(s1[:, :], aPrev[:, :], s0[:, :], 0.0,
                                 AL.mult, AL.add)
            V.tensor_tensor(s2[:, :], SMb[:, 1:T + 1], s1[:, :], AL.add)
            V.tensor_scalar(SMb[:, 1:T + 1], s2[:, :], NEARZERO, cc(C_FC),
                            AL.max, AL.min)
            last_e16 = True
        elif ch == "H":
            # chord Newton: reuse uPrev/aPrev, recompute residual only
            V.tensor_tensor(s3[:, :], SMb[:, 0:T], uPrev[:, :], AL.add)      # SMa
            V.tensor_scalar(s4[:, :], s3[:, :], cc(C_FC), None, AL.min)      # SMmid
            V.tensor_scalar(g5[:, :], s4[:, :], cc(C_ILPFC), 1.0, AL.mult, AL.min)  # q
            V.tensor_tensor(eSoil[:, :], PEh[:, :], g5[:, :], AL.mult)       # e
            V.tensor_tensor(s6[:, :], s4[:, :], eSoil[:, :], AL.subtract)    # fval
            V.tensor_tensor(s0[:, :], s6[:, :], SMb[:, 1:T + 1], AL.subtract)  # rho
            V.tensor_tensor_scan(s1[:, :], aPrev[:, :], s0[:, :], 0.0,
                                 AL.mult, AL.add)
            V.tensor_tensor(s2[:, :], SMb[:, 1:T + 1], s1[:, :], AL.add)
            V.tensor_scalar(SMb[:, 1:T + 1], s2[:, :], NEARZERO, cc(C_FC),
                            AL.max, AL.min)
            last_e16 = True
        elif ch == "P":
            A.activation(s0[:, :], SMb[:, 0:T], AF.Ln)
            A.activation(g1[:, :], s0[:, :], AF.Exp, scale=cc(C_BETA), bias=cc(C_BLIF))
            A.activation(g0[:, :], g1[:, :], AF.Copy, scale=-1.0, bias=1.0)  # 1-sw
            V.tensor_tensor(g6[:, :], g0[:, :], INh[:, :], AL.mult)          # u
            V.tensor_tensor(s3[:, :], SMb[:, 0:T], g6[:, :], AL.add)         # SMa
            V.tensor_scalar(s4[:, :], s3[:, :], cc(C_FC), None, AL.min)      # SMmid
            V.tensor_scalar(g5[:, :], s4[:, :], cc(C_ILPFC), 1.0, AL.mult, AL.min)  # q
            V.tensor_tensor(eSoil[:, :], PEh[:, :], g5[:, :], AL.mult)       # e
            V.tensor_tensor(s6[:, :], s4[:, :], eSoil[:, :], AL.subtract)    # fval
            V.tensor_tensor(s0[:, :], s6[:, :], SMb[:, 1:T + 1], AL.subtract)  # rho
            V.tensor_tensor_scan(s1[:, :], aPrev[:, :], s0[:, :], 0.0,
                                 AL.mult, AL.add)
            V.tensor_tensor(s2[:, :], SMb[:, 1:T + 1], s1[:, :], AL.add)
            V.tensor_scalar(SMb[:, 1:T + 1], s2[:, :], NEARZERO, cc(C_FC),
                            AL.max, AL.min)
            last_e16 = True
        elif ch == "s":
            A.activation(s0[:, :], SMb[:, 0:T], AF.Ln)
            A.activation(s1[:, :], s0[:, :], AF.Exp, scale=cc(C_BETA), bias=cc(C_BLIF))
            A.activation(s2[:, :], s0[:, :], AF.Exp, scale=cc(C_BM1), bias=cc(C_SWPB))
            A.activation(s0[:, :], s1[:, :], AF.Copy, scale=-1.0, bias=1.0)
            V.tensor_tensor(s1[:, :], s0[:, :], INb[:, :], AL.mult)          # u
            V.tensor_tensor(s3[:, :], SMb[:, 0:T], s1[:, :], AL.add)         # SMa
            V.tensor_scalar(s4[:, :], s3[:, :], cc(C_FC), None, AL.min)      # SMmid
            V.tensor_scalar(s5[:, :], s4[:, :], cc(C_ILPFC), 1.0, AL.mult, AL.min)
            V.tensor_tensor(ebF32[:, :], PEp[:, :], s5[:, :], AL.mult)       # e
            V.tensor_tensor(s6[:, :], s4[:, :], ebF32[:, :], AL.subtract)    # fval
            V.tensor_scalar(s0[:, :], s3[:, :], cc(C_FC), None, AL.is_lt)
            V.tensor_tensor(s1[:, :], INb[:, :], s2[:, :], AL.mult)
            A.activation(s1[:, :], s1[:, :], AF.Copy, scale=-1.0, bias=1.0)
            V.tensor_scalar(s2[:, :], s5[:, :], 1.0, None, AL.is_lt)
            V.tensor_tensor(s2[:, :], s2[:, :], PETinv[:, :], AL.mult)
            A.activation(s2[:, :], s2[:, :], AF.Copy, scale=-1.0, bias=1.0)
            V.tensor_tensor(s3[:, :], s0[:, :], s1[:, :], AL.mult)
            V.tensor_tensor(s4[:, :], s3[:, :], s2[:, :], AL.mult)
            V.tensor_scalar(s5[:, :], s4[:, :], 0.0, 1.0, AL.max, AL.min)
            V.tensor_tensor(s0[:, :], s6[:, :], SMb[:, 1:T + 1], AL.subtract)
            V.tensor_tensor_scan(s1[:, :], s5[:, :], s0[:, :], 0.0,
                                 AL.mult, AL.add)
            V.tensor_tensor(s2[:, :], SMb[:, 1:T + 1], s1[:, :], AL.add)
            V.tensor_scalar(SMb[:, 1:T + 1], s2[:, :], NEARZERO, cc(C_FC),
                            AL.max, AL.min)
            last_e16 = False
        else:
            raise ValueError(f"bad soil plan char {ch}")

    # SUZIN = (INb - e) - (SM[1:] - SM[:T])
    SUZIN = tl("SUZIN")
    esrc = eSoil if last_e16 else ebF32
    V.tensor_tensor(s0[:, :], INb[:, :], esrc[:, :], AL.subtract)
    V.tensor_tensor(s1[:, :], SMb[:, 1:T + 1], SMb[:, 0:T], AL.subtract)
    V.tensor_tensor(SUZIN[:, :], s0[:, :], s1[:, :], AL.subtract)

    # ---- SUZ regime iteration: f16 with f32 polish tail ----
    SUZb = tl("SUZb", T + 1); SINP = tl("SINP")
    SUZh = tl("SUZh", T + 1, F16)
    SUZINh = tl("SUZINh", T, F16); SINPh = tl("SINPh", T, F16)
    h0 = tl("h0", T, F16); h2 = tl("h2", T, F16); h3 = tl("h3", T, F16)
    h4 = tl("h4", T, F16); h5 = tl("h5", T, F16); h6 = tl("h6", T, F16)
    V.memset(SUZb[:, 0:1], 0.001)
    V.memset(SUZh[:, 0:1], 0.001)
    A.activation(SINP[:, :], SUZIN[:, :], AF.Identity, bias=cc(C_NPCAP))
    A.activation(SUZINh[:, :], SUZIN[:, :], AF.Copy)
    A.activation(SINPh[:, :], SINP[:, :], AF.Copy)
    for it in range(n_s):
        if it < n_s - tail:
            if it == 0:
                A.activation(h0[:, :], SUZINh[:, :], AF.Identity,
                             bias=c001[:, 0:1])
            else:
                V.tensor_tensor(h0[:, :], SUZh[:, 0:T], SUZINh[:, :], AL.add)
            V.tensor_scalar(h3[:, :], h0[:, :], cc(C_PCAP), cc(C_CA),
                            AL.is_gt, AL.mult)
            V.tensor_scalar(h6[:, :], h0[:, :], cc(C_PCUZ), cc(C_CB),
                            AL.is_gt, AL.mult)
            V.tensor_tensor(h4[:, :], h3[:, :], h6[:, :], AL.add)      # alpha
            V.tensor_tensor(h5[:, :], h4[:, :], SINPh[:, :], AL.mult)
            V.tensor_scalar(h2[:, :], h0[:, :], cc(C_PCUZ), cc(C_C3),
                            AL.is_gt, AL.mult)
            V.tensor_tensor(h3[:, :], h5[:, :], h2[:, :], AL.add)      # beta
            V.tensor_tensor_scan(SUZh[:, 1:T + 1], h4[:, :], h3[:, :], 0.001,
                                 AL.mult, AL.add)
            continue
        prevb = SUZh if it == n_s - tail else SUZb
        V.tensor_tensor(s0[:, :], prevb[:, 0:T], SUZIN[:, :], AL.add)  # S1
        V.tensor_scalar(s3[:, :], s0[:, :], cc(C_PCAP), cc(C_CA),
                        AL.is_gt, AL.mult)
        V.tensor_scalar(s4[:, :], s0[:, :], cc(C_PCUZ), cc(C_CB),
                        AL.is_gt, AL.mult)
        V.tensor_tensor(s5[:, :], s3[:, :], s4[:, :], AL.add)          # alpha
        V.tensor_tensor(s6[:, :], s5[:, :], SINP[:, :], AL.mult)
        V.tensor_scalar(s2[:, :], s0[:, :], cc(C_PCUZ), cc(C_C3),
                        AL.is_gt, AL.mult)
        V.tensor_tensor(s3[:, :], s6[:, :], s2[:, :], AL.add)          # beta
        V.tensor_tensor_scan(SUZb[:, 1:T + 1], s5[:, :], s3[:, :], 0.001,
                             AL.mult, AL.add)

    # ---- post-SUZ / SLZ ----
    V.tensor_tensor(s0[:, :], SUZb[:, 0:T], SUZIN[:, :], AL.add)       # S1
    V.tensor_scalar(s5[:, :], s0[:, :], cc(C_PCAP), cc(C_1K2),
                    AL.min, AL.mult)                                   # (1-K2)*PERC
    A.activation(s2[:, :], s0[:, :], AF.Relu, bias=cc(C_NPCAP))        # S1-PERC
    V.tensor_tensor(s3[:, :], s2[:, :], SUZb[:, 1:T + 1], AL.subtract)  # Q01
    A.activation(s4[:, :], ones[:, :], AF.Copy, scale=cc(C_1K2))
    V.tensor_tensor_scan(s6[:, :], s4[:, :], s5[:, :], 0.001, AL.mult, AL.add)  # SLZ
    A.activation(s0[:, :], s6[:, :], AF.Copy, scale=cc(C_KAP))         # Q2

    # ---- routing conv in fp16 (tap products split V/Scalar) ----
    # (PE- and Pool-engine variants measured slower: Pool tensor ops contend
    # for SBUF with the DVE ~2-3x, and any PE activity downclocks the DVE
    # ~20% chip-wide.)
    QbH = tl("QbH", T + LENF - 1, F16)
    yA, yB, wq0, wq1 = g0, g1, g2, g3
    V.memset(QbH[:, 0:LENF - 1], 0.0)
    V.tensor_tensor(QbH[:, LENF - 1:T + LENF - 1], s3[:, :], s0[:, :], AL.add)  # Q
    base = LENF - 1
    V.tensor_scalar(yA[:, :], QbH[:, base:base + T], cc(C_W0), None, AL.mult)
    src, dst = yA, yB
    for k in range(1, LENF):
        wq = (wq0, wq1)[k % 2]
        if k <= tau:
            A.activation(wq[:, :], QbH[:, base - k:base - k + T], AF.Copy,
                         scale=cc(C_W0 + k))
        else:
            V.tensor_scalar(wq[:, :], QbH[:, base - k:base - k + T],
                            cc(C_W0 + k), None, AL.mult)
        V.tensor_tensor(dst[:, :], src[:, :], wq[:, :], AL.add)
        src, dst = dst, src
    nc.gpsimd.dma_start(dout[ci], src[:, :])  # casts fp16 -> fp32


# ---------------- host orchestration ----------------
_CACHE = {}


def _get_nc(key=None):
    if key is None:
        key = (SLOT_NS, SOIL_PLAN, N_A, SUZ_TAIL, CONV_TAU)
    if key not in _CACHE:
        _CACHE[key] = build_nc(*key)
    return _CACHE[key]


def cell_layout(p, x_phy):
    """Position i (0..G_PAD-1) holds source cell cells[i]; chunk g=i//128 maps
    to core g%8, slot g//8. Hardest cells first so low slots are hard.
    Difficulty = union of two rankings: coarse-sim SUZ residual and the
    persistence heuristic (a cell is hard if either says so)."""
    G = G_FULL
    dsim = difficulty(p, x_phy)
    Pm = x_phy[:, :, 0].mean(axis=0).astype(np.float64)
    PETm = x_phy[:, :, 2].mean(axis=0).astype(np.float64)
    dcrude = (1.0 - p["parK1"]) * (Pm - 0.7 * PETm > p["parPERC"])

    def ranks(d):
        o = np.argsort(-d, kind="stable")
        r = np.empty(G, np.int64)
        r[o] = np.arange(G)
        return r

    runion = np.minimum(ranks(dsim), ranks(dcrude))
    pad = np.arange(G_PAD - G)
    rall = np.concatenate([runion, runion[pad]])
    order = np.argsort(rall, kind="stable")
    cells = np.concatenate([np.arange(G), pad])[order]
    gchunk = np.arange(G_PAD) // P128
    core_of = gchunk % N_CORES
    return cells, core_of


def kernel(x_phy: np.ndarray, parameters: np.ndarray, trace=False):
    x = np.asarray(x_phy, np.float32)
    par_last = np.asarray(parameters)[-1].astype(np.float32)
    Tn, G, _ = x.shape
    assert Tn == T and G == G_FULL
    p = host_params(par_last)
    cells, core_of = cell_layout(p, x)
    p["_forcing_stats"] = (x[:, :, 0].mean(axis=0).astype(np.float64),
                           x[:, :, 2].mean(axis=0).astype(np.float64))
    consts_all = host_consts(p)[cells]
    xg = x[:, cells, :]
    in_maps = []
    per_core = CHUNKS_PER_CORE * P128
    for c in range(N_CORES):
        idx = np.where(core_of == c)[0]
        blk = np.ascontiguousarray(np.moveaxis(xg[:, idx, :], 0, 1))  # [pc, T, 3]
        in_maps.append({
            "pp": np.ascontiguousarray(blk[:, :, 0]).reshape(CHUNKS_PER_CORE, P128, T),
            "tm": np.ascontiguousarray(blk[:, :, 1]).reshape(CHUNKS_PER_CORE, P128, T),
            "pe": np.ascontiguousarray(blk[:, :, 2]).reshape(CHUNKS_PER_CORE, P128, T),
            "cc": np.ascontiguousarray(consts_all[idx]).reshape(CHUNKS_PER_CORE, P128, NCONST),
        })
    nc = _get_nc()
    res = run_bass_kernel_spmd(nc, in_maps, list(range(N_CORES)), trace=trace)
    out = np.empty((T, G), np.float32)
    for c in range(N_CORES):
        idx = np.where(core_of == c)[0]
        ys = res.results[c]["y"].reshape(per_core, T)
        out[:, cells[idx]] = ys.T  # pad duplicates overwrite identically
    if trace:
        return out, res
    return out


# revision 16
# speedup vs baseline: 1.1900x; 1.0435x over previous
"""Bass/Tile HBV kernel for 8 TRN2 NeuronCores.

Bulk reformulation: per chunk of 128 cells (partition dim) x 730 days (free dim),
the HBV recurrences become hardware tensor_tensor_scan instructions plus bulk
elementwise ops; nonlinear buckets are solved by short Picard/Newton iterations
(validated in numpy to converge well below the 2e-2 gate).

v2: engine-balanced instruction stream. The Vector (DVE) engine is the
bottleneck and is SBUF-read-bandwidth bound, so:
 - unary affine ops (scale/bias/relu/exp/ln/copy-cast) run on the Scalar
   engine (activation), including part of the routing-conv tap products;
 - two-ALU-op tensor_scalar fusions replace tensor_tensor pairs wherever a
   per-partition scalar operand allows;
 - the first soil-moisture Newton iteration is linearized around the constant
   FC/2 initial state (host-precomputed coefficients; no Ln/Exp needed);
 - the final soil iteration reuses the previous linearization (chord Newton);
 - the SUZ regime iteration runs in fp16 with a single fp32 polish pass;
 - per-slot iteration counts tuned in a bit-accurate numpy mirror.

Cells are ranked by a host-side difficulty estimate and striped across cores so
each chunk-slot is difficulty-homogeneous; harder slots run more SUZ regime
iterations.

Self-contained: needs numpy + concourse (+ axon TRN2 devices).
"""
import numpy as np

import concourse.bacc as bacc
import concourse.mybir as mybir
from concourse.bass_utils import run_bass_kernel_spmd
from concourse.tile import TileContext

F32 = mybir.dt.float32
F16 = mybir.dt.float16
AL = mybir.AluOpType
AF = mybir.ActivationFunctionType

T = 730
G_FULL = 10000
N_CORES = 8
CHUNKS_PER_CORE = 10
P128 = 128
G_PAD = N_CORES * CHUNKS_PER_CORE * P128  # 10240
LENF = 15
NEARZERO = 1e-5

PHY_BOUNDS = [
    ("parBETA", 1.0, 6.0), ("parFC", 50.0, 1000.0), ("parK0", 0.05, 0.9),
    ("parK1", 0.01, 0.5), ("parK2", 0.001, 0.2), ("parLP", 0.2, 1.0),
    ("parPERC", 0.0, 10.0), ("parUZL", 0.0, 100.0), ("parTT", -2.5, 2.5),
    ("parCFMAX", 0.5, 10.0), ("parCFR", 0.0, 0.1), ("parCWH", 0.0, 0.2),
]
ROUT_A_BOUNDS = (0.0, 2.9)
ROUT_B_BOUNDS = (0.0, 6.5)

# const column indices
(C_TT, C_MS, C_MB, C_RS, C_RB, C_1CWH, C_FC, C_BETA, C_BLIF, C_BM1, C_SWPB,
 C_ILPFC, C_PCAP, C_PCUZ, C_CA, C_CB, C_C3, C_1K2, C_KAP, C_FCH,
 C_USW0, C_NSWP0, C_NPCAP, C_NFCH) = range(24)
C_W0 = 24
NCONST = C_W0 + LENF  # 39

# per-slot config; slot 0 = hardest cells (per host difficulty ranking)
SLOT_NS = (12, 6, 6, 4, 4, 3, 3, 3, 3, 3)
SUZ_TAIL = 1          # f32 polish iterations at the end of the SUZ loop
SOIL_PLAN = "cggg"    # c=const-linearized f16, h=f16 Newton, s=f32 Newton,
                      # H=f16 chord (frozen linearization)
N_A = 1               # snow Picard passes
N_B = 4               # soil iterations (= len(SOIL_PLAN); kept for sim.py)
SOIL_SM0 = "half"     # first-soil-iteration linearization point: equil | half
CONV_TAU = 12         # conv taps whose w*q product runs on the Scalar engine


def _sigmoid(x):
    return 1.0 / (1.0 + np.exp(-x))


def host_params(par_last):
    phy = _sigmoid(par_last[:, :12].astype(np.float64))
    rout = _sigmoid(par_last[:, 12:].astype(np.float64))
    p = {}
    for i, (nm, lo, hi) in enumerate(PHY_BOUNDS):
        p[nm] = lo + phy[:, i] * (hi - lo)
    p["rout_a"] = ROUT_A_BOUNDS[0] + rout[:, 0] * (ROUT_A_BOUNDS[1] - ROUT_A_BOUNDS[0])
    p["rout_b"] = ROUT_B_BOUNDS[0] + rout[:, 1] * (ROUT_B_BOUNDS[1] - ROUT_B_BOUNDS[0])
    return p


def host_consts(p):
    g = len(p["parTT"])
    c = np.zeros((g, NCONST), np.float64)
    TTp = p["parTT"]; CFMAX = p["parCFMAX"]; CFR = p["parCFR"]
    beta = p["parBETA"]; FC = p["parFC"]
    c[:, C_TT] = TTp
    c[:, C_MS] = CFMAX
    c[:, C_MB] = -CFMAX * TTp
    c[:, C_RS] = -CFR * CFMAX
    c[:, C_RB] = CFR * CFMAX * TTp
    c[:, C_1CWH] = 1.0 + p["parCWH"]
    c[:, C_FC] = FC
    c[:, C_BETA] = beta
    lnInvFC = -np.log(FC)
    c[:, C_BLIF] = beta * lnInvFC
    c[:, C_BM1] = beta - 1.0
    c[:, C_SWPB] = beta * lnInvFC + np.log(beta)
    c[:, C_ILPFC] = 1.0 / (p["parLP"] * FC)
    c[:, C_PCAP] = p["parPERC"]
    c[:, C_PCUZ] = p["parPERC"] + p["parUZL"]
    ca = 1.0 - p["parK1"]
    c[:, C_CA] = ca
    c[:, C_CB] = -p["parK0"] * ca
    c[:, C_C3] = ca * p["parK0"] * p["parUZL"]
    c[:, C_1K2] = 1.0 - p["parK2"]
    c[:, C_KAP] = p["parK2"] / (1.0 - p["parK2"])
    # First-soil-iteration linearization/init point SM0: per-cell equilibrium
    # of the soil water balance under mean forcing (host bisection), clamped
    # away from the edges; falls back to FC/2 when no forcing stats given.
    stats = p.get("_forcing_stats")
    if stats is not None and SOIL_SM0 == "equil":
        INm, PETm = stats
        lpfc = p["parLP"] * FC
        lo = np.full_like(FC, 1e-3)
        hi = FC.copy()
        for _ in range(40):
            mid = 0.5 * (lo + hi)
            f = (INm * (1.0 - (mid / FC) ** beta)
                 - PETm * np.minimum(mid / lpfc, 1.0))
            lo = np.where(f > 0, mid, lo)
            hi = np.where(f > 0, hi, mid)
        SM0 = np.clip(0.5 * (lo + hi), 0.02 * FC, 0.98 * FC)
    else:
        SM0 = 0.5 * FC
    c[:, C_FCH] = SM0
    r0 = SM0 / FC
    sw0 = r0 ** beta
    swp0 = (r0 ** (beta - 1.0)) * beta / FC
    c[:, C_USW0] = 1.0 - sw0
    c[:, C_NSWP0] = -swp0
    c[:, C_NPCAP] = -p["parPERC"]
    c[:, C_NFCH] = -c[:, C_FCH]
    aa = np.maximum(p["rout_a"], 0.0) + 0.1
    theta = np.maximum(p["rout_b"], 0.0) + 0.5
    tk = np.arange(LENF, dtype=np.float64) + 0.5
    wv = np.exp((aa[:, None] - 1.0) * np.log(tk)[None, :]
                - tk[None, :] / theta[:, None])
    c[:, C_W0:C_W0 + LENF] = wv / wv.sum(axis=1, keepdims=True)
    return c.astype(np.float32)


def difficulty(p, x_phy, stride=4, k_lo=4, k_hi=9):
    """Per-cell SUZ iteration difficulty: residual between k_lo and k_hi regime
    iterations of a coarse (time-strided) SUZ solve with a proxy inflow."""
    P = x_phy[::stride, :, 0].astype(np.float64)
    PET = x_phy[::stride, :, 2].astype(np.float64)
    SUZIN = np.maximum(P - 0.7 * PET, 0.0)
    Tc, G = SUZIN.shape
    K0 = p["parK0"]; K1 = p["parK1"]; PCAP = p["parPERC"]; UZL = p["parUZL"]
    ca = 1.0 - K1
    SUZ_prev = np.zeros((Tc, G))
    keep = {}
    SUZ = np.zeros((Tc, G))
    for it in range(k_hi):
        S1 = SUZ_prev + SUZIN
        m1 = S1 > PCAP
        m2 = S1 > PCAP + UZL
        alpha = ca * (1.0 - K0 * m2) * m1
        beta = alpha * (SUZIN - PCAP) + (ca * K0 * UZL) * m2
        s = np.zeros(G)
        for t in range(Tc):
            s = alpha[t] * s + beta[t]
            SUZ[t] = s
        if it + 1 in (k_lo, k_hi):
            keep[it + 1] = SUZ.copy()
        SUZ_prev[1:] = SUZ[:-1]
        SUZ_prev[0] = 0.0
    return np.abs(keep[k_hi] - keep[k_lo]).mean(axis=0)


def build_nc(slot_ns=SLOT_NS, soil_plan=SOIL_PLAN, n_a=N_A, suz_tail=SUZ_TAIL,
             conv_tau=CONV_TAU):
    nc = bacc.Bacc("TRN2", target_bir_lowering=False, debug=False,
                   num_devices=N_CORES)
    din = {}
    for nm in ("pp", "tm", "pe"):
        din[nm] = nc.declare_dram_parameter(nm, [CHUNKS_PER_CORE, P128, T], F32,
                                            isOutput=False)
    din["cc"] = nc.declare_dram_parameter("cc", [CHUNKS_PER_CORE, P128, NCONST],
                                          F32, isOutput=False)
    dout = nc.declare_dram_parameter("y", [CHUNKS_PER_CORE, P128, T], F32,
                                     isOutput=True)
    with TileContext(nc) as tc:
        with tc.tile_pool(name="gl", bufs=1) as gpool:
            zeros = gpool.tile([P128, T], F32, name="zeros")
            nc.vector.memset(zeros[:, :], 0.0)
            ones = gpool.tile([P128, T], F32, name="ones")
            nc.vector.memset(ones[:, :], 1.0)
            c001 = gpool.tile([P128, 1], F32, name="c001")
            nc.vector.memset(c001[:, :], 0.001)
            with tc.tile_pool(name="io", bufs=2) as iop, \
                    tc.tile_pool(name="wk", bufs=2) as wk:
                for ci in range(CHUNKS_PER_CORE):
                    _chunk(nc, (iop, wk), din, dout, ci, zeros, ones,
                           c001, n_a, soil_plan, slot_ns[ci], suz_tail,
                           conv_tau)
    nc.compile()
    return nc


def _chunk(nc, pools, din, dout, ci, zeros, ones, c001, n_a, plan, n_s, tail,
           tau):
    iop, wk = pools
    V = nc.vector
    A = nc.scalar
    dma = nc.sync.dma_start

    def tl(tag, w=T, dt=F32):
        return wk.tile([P128, w], dt, tag=tag, name=tag)

    # io planes
    Pp = iop.tile([P128, T], F32, tag="Pp", name="Pp")
    TMp = iop.tile([P128, T], F32, tag="TMp", name="TMp")
    PEp = iop.tile([P128, T], F32, tag="PEp", name="PEp")
    ct = iop.tile([P128, NCONST], F32, tag="ct", name="ct")
    dma(Pp[:, :], din["pp"][ci])
    dma(TMp[:, :], din["tm"][ci])
    dma(PEp[:, :], din["pe"][ci])
    dma(ct[:, :], din["cc"][ci])

    def cc(i):
        return ct[:, i:i + 1]

    # scratch planes
    s0 = tl("s0"); s1 = tl("s1"); s2 = tl("s2"); s3 = tl("s3")
    s4 = tl("s4"); s5 = tl("s5"); s6 = tl("s6")
    g0 = tl("g0", T, F16); g1 = tl("g1", T, F16); g2 = tl("g2", T, F16)
    g3 = tl("g3", T, F16); g4 = tl("g4", T, F16); g5 = tl("g5", T, F16)
    g6 = tl("g6", T, F16); g7 = tl("g7", T, F16)
    PEh = tl("PEh", T, F16)
    PETinv16 = tl("PETinv16", T, F16)
    INh = tl("INh", T, F16)
    eSoil = tl("eSoil", T, F16)      # e of the last soil iteration (f16 plans)
    has_H = "H" in plan
    has_P = "P" in plan
    uPrev = tl("uPrev", T, F16) if has_H else g6   # frozen-linearization carry
    aPrev = tl("aPrev", T, F16) if (has_H or has_P) else g5
    ebF32 = tl("ebF32") if "s" in plan else eSoil  # e of an f32 soil iteration

    # ---- stage 0 ----
    SNOW = tl("SNOW"); Aa = tl("Aa")
    PETinv = tl("PETinv") if "s" in plan else None
    negR = tl("negR") if n_a >= 2 else None
    A.activation(s0[:, :], TMp[:, :], AF.Relu, scale=cc(C_MS), bias=cc(C_MB))  # M
    if n_a >= 2:
        A.activation(s1[:, :], TMp[:, :], AF.Relu, scale=cc(C_RS), bias=cc(C_RB))
        A.activation(negR[:, :], s1[:, :], AF.Copy, scale=-1.0)
    V.tensor_scalar(s2[:, :], TMp[:, :], cc(C_TT), None, AL.is_lt)
    V.tensor_tensor(SNOW[:, :], Pp[:, :], s2[:, :], AL.mult)
    V.tensor_tensor(Aa[:, :], SNOW[:, :], s0[:, :], AL.subtract)
    if "s" in plan:
        A.activation(PETinv[:, :], PEp[:, :], AF.Copy, scale=cc(C_ILPFC))
    A.activation(PETinv16[:, :], PEp[:, :], AF.Copy, scale=cc(C_ILPFC))
    A.activation(PEh[:, :], PEp[:, :], AF.Copy)

    # ---- snow ----
    Xb = tl("Xb"); Wb = tl("Wb", T + 1)
    cbuf = tl("cbuf", T + 1) if n_a >= 2 else None
    negMW = tl("negMW", T + 1) if n_a >= 2 else None
    V.memset(Wb[:, 0:1], 0.002)
    if n_a >= 2:
        V.memset(cbuf[:, 0:1], 0.0)
        V.memset(negMW[:, 0:1], -0.001)
    sp = None
    for it in range(n_a):
        if it == 0:
            V.tensor_tensor_scan(Xb[:, :], Aa[:, :], zeros[:, :], 0.001,
                                 AL.add, AL.max)
            sp = Xb
        else:
            V.tensor_tensor(negMW[:, 1:T + 1], sp[:, :], Wb[:, 1:T + 1],
                            AL.subtract)
            V.scalar_tensor_tensor(s0[:, :], negMW[:, 0:T], 0.0, negR[:, :],
                                   AL.min, AL.max)                       # -r
            V.tensor_tensor_scan(cbuf[:, 1:T + 1], s0[:, :], s0[:, :], 0.0,
                                 AL.add, AL.bypass)                      # -cumsum r
            V.tensor_tensor_scan(Xb[:, :], Aa[:, :], cbuf[:, 0:T], 0.001,
                                 AL.add, AL.max)
            V.tensor_tensor(s1[:, :], Xb[:, :], cbuf[:, 1:T + 1], AL.subtract)
            sp = s1
        A.activation(s2[:, :], sp[:, :], AF.Copy, scale=cc(C_1CWH))
        V.tensor_tensor_scan(Wb[:, 1:T + 1], SNOW[:, :], s2[:, :], 0.002,
                             AL.add, AL.min)
    INb = tl("INb")
    V.tensor_tensor(s0[:, :], Wb[:, 0:T], Wb[:, 1:T + 1], AL.subtract)
    V.tensor_tensor(INb[:, :], s0[:, :], Pp[:, :], AL.add)
    A.activation(INh[:, :], INb[:, :], AF.Copy)

    # ---- soil Newton (per-plan-char iterations) ----
    SMb = tl("SMb", T + 1)
    V.memset(SMb[:, 0:1], 0.001)
    A.activation(SMb[:, 1:T + 1], ones[:, :], AF.Copy, scale=cc(C_FCH))
    last_e16 = True
    for it, ch in enumerate(plan):
        if ch == "c":
            # const linearization around SM = FC/2 (host-precomputed coeffs)
            A.activation(g1[:, :], INh[:, :], AF.Identity, scale=cc(C_USW0),
                         bias=cc(C_FCH))                            # SMa
            V.tensor_scalar(g2[:, :], g1[:, :], cc(C_FC), None, AL.min)  # SMmid
            V.tensor_scalar(g5[:, :], g2[:, :], cc(C_ILPFC), 1.0,
                            AL.mult, AL.min)                        # q
            V.tensor_tensor(eSoil[:, :], PEh[:, :], g5[:, :], AL.mult)   # e
            V.tensor_tensor(g0[:, :], g2[:, :], eSoil[:, :], AL.subtract)  # fval
            V.tensor_scalar(g3[:, :], g1[:, :], cc(C_FC), None, AL.is_lt)  # mFC
            A.activation(g2[:, :], INh[:, :], AF.Identity, scale=cc(C_NSWP0),
                         bias=1.0)                                  # 1-IN*swp0
            V.tensor_scalar(g7[:, :], g5[:, :], 1.0, None, AL.is_lt)     # mEF
            V.tensor_tensor(g4[:, :], g7[:, :], PETinv16[:, :], AL.mult)
            A.activation(g7[:, :], g4[:, :], AF.Copy, scale=-1.0, bias=1.0)
            V.tensor_tensor(g4[:, :], g3[:, :], g2[:, :], AL.mult)
            V.tensor_tensor(g3[:, :], g4[:, :], g7[:, :], AL.mult)
            V.tensor_scalar(aPrev[:, :], g3[:, :], 0.0, 1.0, AL.max, AL.min)  # a
            A.activation(s0[:, :], g0[:, :], AF.Identity, scale=1.0,
                         bias=cc(C_NFCH))                           # rho
            V.tensor_tensor_scan(s1[:, :], aPrev[:, :], s0[:, :], 0.0,
                                 AL.mult, AL.add)
            V.tensor_scalar(s2[:, :], s1[:, :], cc(C_FCH), NEARZERO,
                            AL.add, AL.max)
            V.tensor_scalar(SMb[:, 1:T + 1], s2[:, :], cc(C_FC), None, AL.min)
            if "H" in plan:
                # u carry for a following frozen iteration
                V.tensor_scalar(uPrev[:, :], INh[:, :], cc(C_USW0), None,
                                AL.mult)
            last_e16 = True
        elif ch == "g":
            # f16 Newton, simplified derivative a = clip(mFC*(1-IN*swp), 0, 1)
            A.activation(s0[:, :], SMb[:, 0:T], AF.Ln)
            A.activation(g1[:, :], s0[:, :], AF.Exp, scale=cc(C_BETA), bias=cc(C_BLIF))
            A.activation(g2[:, :], s0[:, :], AF.Exp, scale=cc(C_BM1), bias=cc(C_SWPB))
            A.activation(g0[:, :], g1[:, :], AF.Copy, scale=-1.0, bias=1.0)  # 1-sw
            V.tensor_tensor(g6[:, :], g0[:, :], INh[:, :], AL.mult)          # u
            V.tensor_tensor(s3[:, :], SMb[:, 0:T], g6[:, :], AL.add)         # SMa
            V.tensor_scalar(s4[:, :], s3[:, :], cc(C_FC), None, AL.min)      # SMmid
            V.tensor_scalar(g5[:, :], s4[:, :], cc(C_ILPFC), 1.0, AL.mult, AL.min)  # q
            V.tensor_tensor(eSoil[:, :], PEh[:, :], g5[:, :], AL.mult)       # e
            V.tensor_tensor(s6[:, :], s4[:, :], eSoil[:, :], AL.subtract)    # fval
            V.tensor_scalar(g3[:, :], s3[:, :], cc(C_FC), None, AL.is_lt)    # mFC
            V.tensor_tensor(g4[:, :], INh[:, :], g2[:, :], AL.mult)
            A.activation(g2[:, :], g4[:, :], AF.Relu, scale=-1.0, bias=1.0)  # clip(1-IN*swp,0,·)
            V.tensor_tensor(aPrev[:, :], g3[:, :], g2[:, :], AL.mult)        # a
            V.tensor_tensor(s0[:, :], s6[:, :], SMb[:, 1:T + 1], AL.subtract)  # rho
            V.tensor_tensor_scan(s1[:, :], aPrev[:, :], s0[:, :], 0.0,
                                 AL.mult, AL.add)
            V.tensor_tensor(s2[:, :], SMb[:, 1:T + 1], s1[:, :], AL.add)
            V.tensor_scalar(SMb[:, 1:T + 1], s2[:, :], NEARZERO, cc(C_FC),
                            AL.max, AL.min)
            last_e16 = True
        elif ch == "h":
            A.activation(s0[:, :], SMb[:, 0:T], AF.Ln)
            A.activation(g1[:, :], s0[:, :], AF.Exp, scale=cc(C_BETA), bias=cc(C_BLIF))
            A.activation(g2[:, :], s0[:, :], AF.Exp, scale=cc(C_BM1), bias=cc(C_SWPB))
            A.activation(g0[:, :], g1[:, :], AF.Copy, scale=-1.0, bias=1.0)  # 1-sw
            V.tensor_tensor(uPrev[:, :], g0[:, :], INh[:, :], AL.mult)       # u
            V.tensor_tensor(s3[:, :], SMb[:, 0:T], uPrev[:, :], AL.add)      # SMa
            V.tensor_scalar(s4[:, :], s3[:, :], cc(C_FC), None, AL.min)      # SMmid
            V.tensor_scalar(g5[:, :], s4[:, :], cc(C_ILPFC), 1.0, AL.mult, AL.min)  # q
            V.tensor_tensor(eSoil[:, :], PEh[:, :], g5[:, :], AL.mult)       # e
            V.tensor_tensor(s6[:, :], s4[:, :], eSoil[:, :], AL.subtract)    # fval
            V.tensor_scalar(g3[:, :], s3[:, :], cc(C_FC), None, AL.is_lt)    # mFC
            V.tensor_tensor(g4[:, :], INh[:, :], g2[:, :], AL.mult)
            A.activation(g2[:, :], g4[:, :], AF.Copy, scale=-1.0, bias=1.0)  # 1-IN*swp
            V.tensor_scalar(g7[:, :], g5[:, :], 1.0, None, AL.is_lt)         # mEF
            V.tensor_tensor(g4[:, :], g7[:, :], PETinv16[:, :], AL.mult)
            A.activation(g7[:, :], g4[:, :], AF.Copy, scale=-1.0, bias=1.0)
            V.tensor_tensor(g4[:, :], g3[:, :], g2[:, :], AL.mult)
            V.tensor_tensor(g3[:, :], g4[:, :], g7[:, :], AL.mult)
            V.tensor_scalar(aPrev[:, :], g3[:, :], 0.0, 1.0, AL.max, AL.min)  # a
            V.tensor_tensor(s0[:, :], s6[:, :], SMb[:, 1:T + 1], AL.subtract)  # rho
            V.tensor_tensor_scan(s1[:, :], aPrev[:, :], s0[:, :], 0.0,
                                 AL.mult, AL.add)
            V.tensor_tensor(s2[:, :], SMb[:, 1:T + 1], s1[:, :], AL.add)
            V.tensor_scalar(SMb[:, 1:T + 1], s2[:, :], NEARZERO, cc(C_FC),
                            AL.max, AL.min)
            last_e16 = True
        elif ch == "H":
            # chord Newton: reuse uPrev/aPrev, recompute residual only
            V.tensor_tensor(s3[:, :], SMb[:, 0:T], uPrev[:, :], AL.add)      # SMa
            V.tensor_scalar(s4[:, :], s3[:, :], cc(C_FC), None, AL.min)      # SMmid
            V.tensor_scalar(g5[:, :], s4[:, :], cc(C_ILPFC), 1.0, AL.mult, AL.min)  # q
            V.tensor_tensor(eSoil[:, :], PEh[:, :], g5[:, :], AL.mult)       # e
            V.tensor_tensor(s6[:, :], s4[:, :], eSoil[:, :], AL.subtract)    # fval
            V.tensor_tensor(s0[:, :], s6[:, :], SMb[:, 1:T + 1], AL.subtract)  # rho
            V.tensor_tensor_scan(s1[:, :], aPrev[:, :], s0[:, :], 0.0,
                                 AL.mult, AL.add)
            V.tensor_tensor(s2[:, :], SMb[:, 1:T + 1], s1[:, :], AL.add)
            V.tensor_scalar(SMb[:, 1:T + 1], s2[:, :], NEARZERO, cc(C_FC),
                            AL.max, AL.min)
            last_e16 = True
        elif ch == "P":
            A.activation(s0[:, :], SMb[:, 0:T], AF.Ln)
            A.activation(g1[:, :], s0[:, :], AF.Exp, scale=cc(C_BETA), bias=cc(C_BLIF))
            A.activation(g0[:, :], g1[:, :], AF.Copy, scale=-1.0, bias=1.0)  # 1-sw
            V.tensor_tensor(g6[:, :], g0[:, :], INh[:, :], AL.mult)          # u
            V.tensor_tensor(s3[:, :], SMb[:, 0:T], g6[:, :], AL.add)         # SMa
            V.tensor_scalar(s4[:, :], s3[:, :], cc(C_FC), None, AL.min)      # SMmid
            V.tensor_scalar(g5[:, :], s4[:, :], cc(C_ILPFC), 1.0, AL.mult, AL.min)  # q
            V.tensor_tensor(eSoil[:, :], PEh[:, :], g5[:, :], AL.mult)       # e
            V.tensor_tensor(s6[:, :], s4[:, :], eSoil[:, :], AL.subtract)    # fval
            V.tensor_tensor(s0[:, :], s6[:, :], SMb[:, 1:T + 1], AL.subtract)  # rho
            V.tensor_tensor_scan(s1[:, :], aPrev[:, :], s0[:, :], 0.0,
                                 AL.mult, AL.add)
            V.tensor_tensor(s2[:, :], SMb[:, 1:T + 1], s1[:, :], AL.add)
            V.tensor_scalar(SMb[:, 1:T + 1], s2[:, :], NEARZERO, cc(C_FC),
                            AL.max, AL.min)
            last_e16 = True
        elif ch == "s":
            A.activation(s0[:, :], SMb[:, 0:T], AF.Ln)
            A.activation(s1[:, :], s0[:, :], AF.Exp, scale=cc(C_BETA), bias=cc(C_BLIF))
            A.activation(s2[:, :], s0[:, :], AF.Exp, scale=cc(C_BM1), bias=cc(C_SWPB))
            A.activation(s0[:, :], s1[:, :], AF.Copy, scale=-1.0, bias=1.0)
            V.tensor_tensor(s1[:, :], s0[:, :], INb[:, :], AL.mult)          # u
            V.tensor_tensor(s3[:, :], SMb[:, 0:T], s1[:, :], AL.add)         # SMa
            V.tensor_scalar(s4[:, :], s3[:, :], cc(C_FC), None, AL.min)      # SMmid
            V.tensor_scalar(s5[:, :], s4[:, :], cc(C_ILPFC), 1.0, AL.mult, AL.min)
            V.tensor_tensor(ebF32[:, :], PEp[:, :], s5[:, :], AL.mult)       # e
            V.tensor_tensor(s6[:, :], s4[:, :], ebF32[:, :], AL.subtract)    # fval
            V.tensor_scalar(s0[:, :], s3[:, :], cc(C_FC), None, AL.is_lt)
            V.tensor_tensor(s1[:, :], INb[:, :], s2[:, :], AL.mult)
            A.activation(s1[:, :], s1[:, :], AF.Copy, scale=-1.0, bias=1.0)
            V.tensor_scalar(s2[:, :], s5[:, :], 1.0, None, AL.is_lt)
            V.tensor_tensor(s2[:, :], s2[:, :], PETinv[:, :], AL.mult)
            A.activation(s2[:, :], s2[:, :], AF.Copy, scale=-1.0, bias=1.0)
            V.tensor_tensor(s3[:, :], s0[:, :], s1[:, :], AL.mult)
            V.tensor_tensor(s4[:, :], s3[:, :], s2[:, :], AL.mult)
            V.tensor_scalar(s5[:, :], s4[:, :], 0.0, 1.0, AL.max, AL.min)
            V.tensor_tensor(s0[:, :], s6[:, :], SMb[:, 1:T + 1], AL.subtract)
            V.tensor_tensor_scan(s1[:, :], s5[:, :], s0[:, :], 0.0,
                                 AL.mult, AL.add)
            V.tensor_tensor(s2[:, :], SMb[:, 1:T + 1], s1[:, :], AL.add)
            V.tensor_scalar(SMb[:, 1:T + 1], s2[:, :], NEARZERO, cc(C_FC),
                            AL.max, AL.min)
            last_e16 = False
        else:
            raise ValueError(f"bad soil plan char {ch}")

    # SUZIN = (INb - e) - (SM[1:] - SM[:T])
    SUZIN = tl("SUZIN")
    esrc = eSoil if last_e16 else ebF32
    V.tensor_tensor(s0[:, :], INb[:, :], esrc[:, :], AL.subtract)
    V.tensor_tensor(s1[:, :], SMb[:, 1:T + 1], SMb[:, 0:T], AL.subtract)
    V.tensor_tensor(SUZIN[:, :], s0[:, :], s1[:, :], AL.subtract)

    # ---- SUZ regime iteration: f16 with f32 polish tail ----
    SUZb = tl("SUZb", T + 1); SINP = tl("SINP")
    SUZh = tl("SUZh", T + 1, F16)
    SUZINh = tl("SUZINh", T, F16); SINPh = tl("SINPh", T, F16)
    h0 = tl("h0", T, F16); h2 = tl("h2", T, F16); h3 = tl("h3", T, F16)
    h4 = tl("h4", T, F16); h5 = tl("h5", T, F16); h6 = tl("h6", T, F16)
    V.memset(SUZb[:, 0:1], 0.001)
    V.memset(SUZh[:, 0:1], 0.001)
    A.activation(SINP[:, :], SUZIN[:, :], AF.Identity, bias=cc(C_NPCAP))
    A.activation(SUZINh[:, :], SUZIN[:, :], AF.Copy)
    A.activation(SINPh[:, :], SINP[:, :], AF.Copy)
    for it in range(n_s):
        if it < n_s - tail:
            if it == 0:
                A.activation(h0[:, :], SUZINh[:, :], AF.Identity,
                             bias=c001[:, 0:1])
            else:
                V.tensor_tensor(h0[:, :], SUZh[:, 0:T], SUZINh[:, :], AL.add)
            V.tensor_scalar(h3[:, :], h0[:, :], cc(C_PCAP), cc(C_CA),
                            AL.is_gt, AL.mult)
            V.tensor_scalar(h6[:, :], h0[:, :], cc(C_PCUZ), cc(C_CB),
                            AL.is_gt, AL.mult)
            V.tensor_tensor(h4[:, :], h3[:, :], h6[:, :], AL.add)      # alpha
            V.tensor_tensor(h5[:, :], h4[:, :], SINPh[:, :], AL.mult)
            V.tensor_scalar(h2[:, :], h0[:, :], cc(C_PCUZ), cc(C_C3),
                            AL.is_gt, AL.mult)
            V.tensor_tensor(h3[:, :], h5[:, :], h2[:, :], AL.add)      # beta
            V.tensor_tensor_scan(SUZh[:, 1:T + 1], h4[:, :], h3[:, :], 0.001,
                                 AL.mult, AL.add)
            continue
        prevb = SUZh if it == n_s - tail else SUZb
        V.tensor_tensor(s0[:, :], prevb[:, 0:T], SUZIN[:, :], AL.add)  # S1
        V.tensor_scalar(s3[:, :], s0[:, :], cc(C_PCAP), cc(C_CA),
                        AL.is_gt, AL.mult)
        V.tensor_scalar(s4[:, :], s0[:, :], cc(C_PCUZ), cc(C_CB),
                        AL.is_gt, AL.mult)
        V.tensor_tensor(s5[:, :], s3[:, :], s4[:, :], AL.add)          # alpha
        V.tensor_tensor(s6[:, :], s5[:, :], SINP[:, :], AL.mult)
        V.tensor_scalar(s2[:, :], s0[:, :], cc(C_PCUZ), cc(C_C3),
                        AL.is_gt, AL.mult)
        V.tensor_tensor(s3[:, :], s6[:, :], s2[:, :], AL.add)          # beta
        V.tensor_tensor_scan(SUZb[:, 1:T + 1], s5[:, :], s3[:, :], 0.001,
                             AL.mult, AL.add)

    # ---- post-SUZ / SLZ ----
    V.tensor_tensor(s0[:, :], SUZb[:, 0:T], SUZIN[:, :], AL.add)       # S1
    V.tensor_scalar(s5[:, :], s0[:, :], cc(C_PCAP), cc(C_1K2),
                    AL.min, AL.mult)                                   # (1-K2)*PERC
    A.activation(s2[:, :], s0[:, :], AF.Relu, bias=cc(C_NPCAP))        # S1-PERC
    V.tensor_tensor(s3[:, :], s2[:, :], SUZb[:, 1:T + 1], AL.subtract)  # Q01
    A.activation(s4[:, :], ones[:, :], AF.Copy, scale=cc(C_1K2))
    V.tensor_tensor_scan(s6[:, :], s4[:, :], s5[:, :], 0.001, AL.mult, AL.add)  # SLZ
    A.activation(s0[:, :], s6[:, :], AF.Copy, scale=cc(C_KAP))         # Q2

    # ---- routing conv in fp16 (tap products split V/Scalar) ----
    # (PE- and Pool-engine variants measured slower: Pool tensor ops contend
    # for SBUF with the DVE ~2-3x, and any PE activity downclocks the DVE
    # ~20% chip-wide.)
    QbH = tl("QbH", T + LENF - 1, F16)
    yA, yB, wq0, wq1 = g0, g1, g2, g3
    V.memset(QbH[:, 0:LENF - 1], 0.0)
    V.tensor_tensor(QbH[:, LENF - 1:T + LENF - 1], s3[:, :], s0[:, :], AL.add)  # Q
    base = LENF - 1
    A.activation(yA[:, :], QbH[:, base:base + T], AF.Copy, scale=cc(C_W0))
    src, dst = yA, yB
    for k in range(1, LENF):
        wq = (wq0, wq1)[k % 2]
        if k <= tau:
            A.activation(wq[:, :], QbH[:, base - k:base - k + T], AF.Copy,
                         scale=cc(C_W0 + k))
        else:
            V.tensor_scalar(wq[:, :], QbH[:, base - k:base - k + T],
                            cc(C_W0 + k), None, AL.mult)
        V.tensor_tensor(dst[:, :], src[:, :], wq[:, :], AL.add)
        src, dst = dst, src
    nc.gpsimd.dma_start(dout[ci], src[:, :])  # casts fp16 -> fp32


# ---------------- host orchestration ----------------
_CACHE = {}


def _get_nc(key=None):
    if key is None:
        key = (SLOT_NS, SOIL_PLAN, N_A, SUZ_TAIL, CONV_TAU)
    if key not in _CACHE:
        _CACHE[key] = build_nc(*key)
    return _CACHE[key]


def cell_layout(p, x_phy):
    """Position i (0..G_PAD-1) holds source cell cells[i]; chunk g=i//128 maps
    to core g%8, slot g//8. Hardest cells first so low slots are hard.
    Difficulty = union of two rankings: coarse-sim SUZ residual and the
    persistence heuristic (a cell is hard if either says so)."""
    G = G_FULL
    dsim = difficulty(p, x_phy)
    Pm = x_phy[:, :, 0].mean(axis=0).astype(np.float64)
    PETm = x_phy[:, :, 2].mean(axis=0).astype(np.float64)
    dcrude = (1.0 - p["parK1"]) * (Pm - 0.7 * PETm > p["parPERC"])

    def ranks(d):
        o = np.argsort(-d, kind="stable")
        r = np.empty(G, np.int64)
        r[o] = np.arange(G)
        return r

    runion = np.minimum(ranks(dsim), ranks(dcrude))
    pad = np.arange(G_PAD - G)
    rall = np.concatenate([runion, runion[pad]])
    order = np.argsort(rall, kind="stable")
    cells = np.concatenate([np.arange(G), pad])[order]
    gchunk = np.arange(G_PAD) // P128
    core_of = gchunk % N_CORES
    return cells, core_of


def kernel(x_phy: np.ndarray, parameters: np.ndarray, trace=False):
    x = np.asarray(x_phy, np.float32)
    par_last = np.asarray(parameters)[-1].astype(np.float32)
    Tn, G, _ = x.shape
    assert Tn == T and G == G_FULL
    p = host_params(par_last)
    cells, core_of = cell_layout(p, x)
    p["_forcing_stats"] = (x[:, :, 0].mean(axis=0).astype(np.float64),
                           x[:, :, 2].mean(axis=0).astype(np.float64))
    consts_all = host_consts(p)[cells]
    xg = x[:, cells, :]
    in_maps = []
    per_core = CHUNKS_PER_CORE * P128
    for c in range(N_CORES):
        idx = np.where(core_of == c)[0]
        blk = np.ascontiguousarray(np.moveaxis(xg[:, idx, :], 0, 1))  # [pc, T, 3]
        in_maps.append({
            "pp": np.ascontiguousarray(blk[:, :, 0]).reshape(CHUNKS_PER_CORE, P128, T),
            "tm": np.ascontiguousarray(blk[:, :, 1]).reshape(CHUNKS_PER_CORE, P128, T),
            "pe": np.ascontiguousarray(blk[:, :, 2]).reshape(CHUNKS_PER_CORE, P128, T),
            "cc": np.ascontiguousarray(consts_all[idx]).reshape(CHUNKS_PER_CORE, P128, NCONST),
        })
    nc = _get_nc()
    res = run_bass_kernel_spmd(nc, in_maps, list(range(N_CORES)), trace=trace)
    out = np.empty((T, G), np.float32)
    for c in range(N_CORES):
        idx = np.where(core_of == c)[0]
        ys = res.results[c]["y"].reshape(per_core, T)
        out[:, cells[idx]] = ys.T  # pad duplicates overwrite identically
    if trace:
        return out, res
    return out


# revision 18
# speedup vs baseline: 1.2129x; 1.0193x over previous
"""Bass/Tile HBV kernel for 8 TRN2 NeuronCores.

Bulk reformulation: per chunk of 128 cells (partition dim) x 730 days (free dim),
the HBV recurrences become hardware tensor_tensor_scan instructions plus bulk
elementwise ops; nonlinear buckets are solved by short Picard/Newton iterations
(validated in numpy to converge well below the 2e-2 gate).

v2: engine-balanced instruction stream. The Vector (DVE) engine is the
bottleneck and is SBUF-read-bandwidth bound, so:
 - unary affine ops (scale/bias/relu/exp/ln/copy-cast) run on the Scalar
   engine (activation), including part of the routing-conv tap products;
 - two-ALU-op tensor_scalar fusions replace tensor_tensor pairs wherever a
   per-partition scalar operand allows;
 - the first soil-moisture Newton iteration is linearized around the constant
   FC/2 initial state (host-precomputed coefficients; no Ln/Exp needed);
 - the final soil iteration reuses the previous linearization (chord Newton);
 - the SUZ regime iteration runs in fp16 with a single fp32 polish pass;
 - per-slot iteration counts tuned in a bit-accurate numpy mirror.

Cells are ranked by a host-side difficulty estimate and striped across cores so
each chunk-slot is difficulty-homogeneous; harder slots run more SUZ regime
iterations.

Self-contained: needs numpy + concourse (+ axon TRN2 devices).
"""
import numpy as np

import concourse.bacc as bacc
import concourse.mybir as mybir
from concourse.bass_utils import run_bass_kernel_spmd
from concourse.tile import TileContext

F32 = mybir.dt.float32
F16 = mybir.dt.float16
AL = mybir.AluOpType
AF = mybir.ActivationFunctionType

T = 730
G_FULL = 10000
N_CORES = 8
CHUNKS_PER_CORE = 10
P128 = 128
G_PAD = N_CORES * CHUNKS_PER_CORE * P128  # 10240
LENF = 15
NEARZERO = 1e-5

PHY_BOUNDS = [
    ("parBETA", 1.0, 6.0), ("parFC", 50.0, 1000.0), ("parK0", 0.05, 0.9),
    ("parK1", 0.01, 0.5), ("parK2", 0.001, 0.2), ("parLP", 0.2, 1.0),
    ("parPERC", 0.0, 10.0), ("parUZL", 0.0, 100.0), ("parTT", -2.5, 2.5),
    ("parCFMAX", 0.5, 10.0), ("parCFR", 0.0, 0.1), ("parCWH", 0.0, 0.2),
]
ROUT_A_BOUNDS = (0.0, 2.9)
ROUT_B_BOUNDS = (0.0, 6.5)

# const column indices
(C_TT, C_MS, C_MB, C_RS, C_RB, C_1CWH, C_FC, C_BETA, C_BLIF, C_BM1, C_SWPB,
 C_ILPFC, C_PCAP, C_PCUZ, C_CA, C_CB, C_C3, C_1K2, C_KAP, C_FCH,
 C_USW0, C_NSWP0, C_NPCAP, C_NFCH) = range(24)
C_W0 = 24
NCONST = C_W0 + LENF  # 39

# per-slot config; slot 0 = hardest cells (per host difficulty ranking)
SLOT_NS = (12, 6, 6, 4, 4, 3, 3, 3, 3, 3)
SUZ_TAIL = 1          # f32 polish iterations at the end of the SUZ loop
SOIL_PLAN = "cggg"    # c=const-linearized f16, h=f16 Newton, s=f32 Newton,
                      # H=f16 chord (frozen linearization)
N_A = 1               # snow Picard passes
N_B = 4               # soil iterations (= len(SOIL_PLAN); kept for sim.py)
SOIL_SM0 = "half"     # first-soil-iteration linearization point: equil | half
CONV_TAU = 0          # deferred conv taps run V-only (they fill ACT-latency gaps)


def _sigmoid(x):
    return 1.0 / (1.0 + np.exp(-x))


def host_params(par_last):
    phy = _sigmoid(par_last[:, :12].astype(np.float64))
    rout = _sigmoid(par_last[:, 12:].astype(np.float64))
    p = {}
    for i, (nm, lo, hi) in enumerate(PHY_BOUNDS):
        p[nm] = lo + phy[:, i] * (hi - lo)
    p["rout_a"] = ROUT_A_BOUNDS[0] + rout[:, 0] * (ROUT_A_BOUNDS[1] - ROUT_A_BOUNDS[0])
    p["rout_b"] = ROUT_B_BOUNDS[0] + rout[:, 1] * (ROUT_B_BOUNDS[1] - ROUT_B_BOUNDS[0])
    return p


def host_consts(p):
    g = len(p["parTT"])
    c = np.zeros((g, NCONST), np.float64)
    TTp = p["parTT"]; CFMAX = p["parCFMAX"]; CFR = p["parCFR"]
    beta = p["parBETA"]; FC = p["parFC"]
    c[:, C_TT] = TTp
    c[:, C_MS] = CFMAX
    c[:, C_MB] = -CFMAX * TTp
    c[:, C_RS] = -CFR * CFMAX
    c[:, C_RB] = CFR * CFMAX * TTp
    c[:, C_1CWH] = 1.0 + p["parCWH"]
    c[:, C_FC] = FC
    c[:, C_BETA] = beta
    lnInvFC = -np.log(FC)
    c[:, C_BLIF] = beta * lnInvFC
    c[:, C_BM1] = beta - 1.0
    c[:, C_SWPB] = beta * lnInvFC + np.log(beta)
    c[:, C_ILPFC] = 1.0 / (p["parLP"] * FC)
    c[:, C_PCAP] = p["parPERC"]
    c[:, C_PCUZ] = p["parPERC"] + p["parUZL"]
    ca = 1.0 - p["parK1"]
    c[:, C_CA] = ca
    c[:, C_CB] = -p["parK0"] * ca
    c[:, C_C3] = ca * p["parK0"] * p["parUZL"]
    c[:, C_1K2] = 1.0 - p["parK2"]
    c[:, C_KAP] = p["parK2"] / (1.0 - p["parK2"])
    # First-soil-iteration linearization/init point SM0: per-cell equilibrium
    # of the soil water balance under mean forcing (host bisection), clamped
    # away from the edges; falls back to FC/2 when no forcing stats given.
    stats = p.get("_forcing_stats")
    if stats is not None and SOIL_SM0 == "equil":
        INm, PETm = stats
        lpfc = p["parLP"] * FC
        lo = np.full_like(FC, 1e-3)
        hi = FC.copy()
        for _ in range(40):
            mid = 0.5 * (lo + hi)
            f = (INm * (1.0 - (mid / FC) ** beta)
                 - PETm * np.minimum(mid / lpfc, 1.0))
            lo = np.where(f > 0, mid, lo)
            hi = np.where(f > 0, hi, mid)
        SM0 = np.clip(0.5 * (lo + hi), 0.02 * FC, 0.98 * FC)
    else:
        SM0 = 0.5 * FC
    c[:, C_FCH] = SM0
    r0 = SM0 / FC
    sw0 = r0 ** beta
    swp0 = (r0 ** (beta - 1.0)) * beta / FC
    c[:, C_USW0] = 1.0 - sw0
    c[:, C_NSWP0] = -swp0
    c[:, C_NPCAP] = -p["parPERC"]
    c[:, C_NFCH] = -c[:, C_FCH]
    aa = np.maximum(p["rout_a"], 0.0) + 0.1
    theta = np.maximum(p["rout_b"], 0.0) + 0.5
    tk = np.arange(LENF, dtype=np.float64) + 0.5
    wv = np.exp((aa[:, None] - 1.0) * np.log(tk)[None, :]
                - tk[None, :] / theta[:, None])
    c[:, C_W0:C_W0 + LENF] = wv / wv.sum(axis=1, keepdims=True)
    return c.astype(np.float32)


def difficulty(p, x_phy, stride=4, k_lo=4, k_hi=9):
    """Per-cell SUZ iteration difficulty: residual between k_lo and k_hi regime
    iterations of a coarse (time-strided) SUZ solve with a proxy inflow."""
    P = x_phy[::stride, :, 0].astype(np.float64)
    PET = x_phy[::stride, :, 2].astype(np.float64)
    SUZIN = np.maximum(P - 0.7 * PET, 0.0)
    Tc, G = SUZIN.shape
    K0 = p["parK0"]; K1 = p["parK1"]; PCAP = p["parPERC"]; UZL = p["parUZL"]
    ca = 1.0 - K1
    SUZ_prev = np.zeros((Tc, G))
    keep = {}
    SUZ = np.zeros((Tc, G))
    for it in range(k_hi):
        S1 = SUZ_prev + SUZIN
        m1 = S1 > PCAP
        m2 = S1 > PCAP + UZL
        alpha = ca * (1.0 - K0 * m2) * m1
        beta = alpha * (SUZIN - PCAP) + (ca * K0 * UZL) * m2
        s = np.zeros(G)
        for t in range(Tc):
            s = alpha[t] * s + beta[t]
            SUZ[t] = s
        if it + 1 in (k_lo, k_hi):
            keep[it + 1] = SUZ.copy()
        SUZ_prev[1:] = SUZ[:-1]
        SUZ_prev[0] = 0.0
    return np.abs(keep[k_hi] - keep[k_lo]).mean(axis=0)


def build_nc(slot_ns=SLOT_NS, soil_plan=SOIL_PLAN, n_a=N_A, suz_tail=SUZ_TAIL,
             conv_tau=CONV_TAU):
    nc = bacc.Bacc("TRN2", target_bir_lowering=False, debug=False,
                   num_devices=N_CORES)
    din = {}
    for nm in ("pp", "tm", "pe"):
        din[nm] = nc.declare_dram_parameter(nm, [CHUNKS_PER_CORE, P128, T], F32,
                                            isOutput=False)
    din["cc"] = nc.declare_dram_parameter("cc", [CHUNKS_PER_CORE, P128, NCONST],
                                          F32, isOutput=False)
    dout = nc.declare_dram_parameter("y", [CHUNKS_PER_CORE, P128, T], F32,
                                     isOutput=True)
    with TileContext(nc) as tc:
        with tc.tile_pool(name="gl", bufs=1) as gpool:
            zeros = gpool.tile([P128, T], F32, name="zeros")
            nc.vector.memset(zeros[:, :], 0.0)
            ones = gpool.tile([P128, T], F32, name="ones")
            nc.vector.memset(ones[:, :], 1.0)
            c001 = gpool.tile([P128, 1], F32, name="c001")
            nc.vector.memset(c001[:, :], 0.001)
            with tc.tile_pool(name="io", bufs=2) as iop, \
                    tc.tile_pool(name="wk", bufs=2) as wk:
                filler = []
                for ci in range(CHUNKS_PER_CORE):
                    filler = _chunk(nc, (iop, wk), din, dout, ci, zeros, ones,
                                    c001, n_a, soil_plan, slot_ns[ci],
                                    suz_tail, conv_tau, filler)
                for fn in filler:
                    fn()
    nc.compile()
    return nc


def _chunk(nc, pools, din, dout, ci, zeros, ones, c001, n_a, plan, n_s, tail,
           tau, filler):
    iop, wk = pools

    def fill(n):
        # emit up to n deferred conv ops of the previous chunk; they overlap
        # the Scalar-engine Ln/Exp chains this chunk is about to wait on
        for _ in range(min(n, len(filler))):
            filler.pop(0)()
    V = nc.vector
    A = nc.scalar
    dma = nc.sync.dma_start

    def tl(tag, w=T, dt=F32):
        return wk.tile([P128, w], dt, tag=tag, name=tag)

    # io planes
    Pp = iop.tile([P128, T], F32, tag="Pp", name="Pp")
    TMp = iop.tile([P128, T], F32, tag="TMp", name="TMp")
    PEp = iop.tile([P128, T], F32, tag="PEp", name="PEp")
    ct = iop.tile([P128, NCONST], F32, tag="ct", name="ct")
    dma(Pp[:, :], din["pp"][ci])
    dma(TMp[:, :], din["tm"][ci])
    dma(PEp[:, :], din["pe"][ci])
    dma(ct[:, :], din["cc"][ci])

    def cc(i):
        return ct[:, i:i + 1]

    # scratch planes
    s0 = tl("s0"); s1 = tl("s1"); s2 = tl("s2"); s3 = tl("s3")
    s4 = tl("s4"); s5 = tl("s5"); s6 = tl("s6")
    g0 = tl("g0", T, F16); g1 = tl("g1", T, F16); g2 = tl("g2", T, F16)
    g3 = tl("g3", T, F16); g4 = tl("g4", T, F16); g5 = tl("g5", T, F16)
    g6 = tl("g6", T, F16); g7 = tl("g7", T, F16)
    PEh = tl("PEh", T, F16)
    PETinv16 = tl("PETinv16", T, F16)
    INh = tl("INh", T, F16)
    eSoil = tl("eSoil", T, F16)      # e of the last soil iteration (f16 plans)
    has_H = "H" in plan
    has_P = "P" in plan
    uPrev = tl("uPrev", T, F16) if has_H else g6   # frozen-linearization carry
    aPrev = tl("aPrev", T, F16) if (has_H or has_P) else g5
    ebF32 = tl("ebF32") if "s" in plan else eSoil  # e of an f32 soil iteration

    # ---- stage 0 ----
    SNOW = tl("SNOW"); Aa = tl("Aa")
    PETinv = tl("PETinv") if "s" in plan else None
    negR = tl("negR") if n_a >= 2 else None
    A.activation(s0[:, :], TMp[:, :], AF.Relu, scale=cc(C_MS), bias=cc(C_MB))  # M
    if n_a >= 2:
        A.activation(s1[:, :], TMp[:, :], AF.Relu, scale=cc(C_RS), bias=cc(C_RB))
        A.activation(negR[:, :], s1[:, :], AF.Copy, scale=-1.0)
    V.tensor_scalar(s2[:, :], TMp[:, :], cc(C_TT), None, AL.is_lt)
    V.tensor_tensor(SNOW[:, :], Pp[:, :], s2[:, :], AL.mult)
    V.tensor_tensor(Aa[:, :], SNOW[:, :], s0[:, :], AL.subtract)
    if "s" in plan:
        A.activation(PETinv[:, :], PEp[:, :], AF.Copy, scale=cc(C_ILPFC))
    A.activation(PETinv16[:, :], PEp[:, :], AF.Copy, scale=cc(C_ILPFC))
    A.activation(PEh[:, :], PEp[:, :], AF.Copy)

    # ---- snow ----
    Xb = tl("Xb"); Wb = tl("Wb", T + 1)
    cbuf = tl("cbuf", T + 1) if n_a >= 2 else None
    negMW = tl("negMW", T + 1) if n_a >= 2 else None
    V.memset(Wb[:, 0:1], 0.002)
    if n_a >= 2:
        V.memset(cbuf[:, 0:1], 0.0)
        V.memset(negMW[:, 0:1], -0.001)
    sp = None
    for it in range(n_a):
        if it == 0:
            V.tensor_tensor_scan(Xb[:, :], Aa[:, :], zeros[:, :], 0.001,
                                 AL.add, AL.max)
            sp = Xb
        else:
            V.tensor_tensor(negMW[:, 1:T + 1], sp[:, :], Wb[:, 1:T + 1],
                            AL.subtract)
            V.scalar_tensor_tensor(s0[:, :], negMW[:, 0:T], 0.0, negR[:, :],
                                   AL.min, AL.max)                       # -r
            V.tensor_tensor_scan(cbuf[:, 1:T + 1], s0[:, :], s0[:, :], 0.0,
                                 AL.add, AL.bypass)                      # -cumsum r
            V.tensor_tensor_scan(Xb[:, :], Aa[:, :], cbuf[:, 0:T], 0.001,
                                 AL.add, AL.max)
            V.tensor_tensor(s1[:, :], Xb[:, :], cbuf[:, 1:T + 1], AL.subtract)
            sp = s1
        A.activation(s2[:, :], sp[:, :], AF.Copy, scale=cc(C_1CWH))
        V.tensor_tensor_scan(Wb[:, 1:T + 1], SNOW[:, :], s2[:, :], 0.002,
                             AL.add, AL.min)
    INb = tl("INb")
    V.tensor_tensor(s0[:, :], Wb[:, 0:T], Wb[:, 1:T + 1], AL.subtract)
    V.tensor_tensor(INb[:, :], s0[:, :], Pp[:, :], AL.add)
    A.activation(INh[:, :], INb[:, :], AF.Copy)
    fill(3)

    # ---- soil Newton (per-plan-char iterations) ----
    SMb = tl("SMb", T + 1)
    V.memset(SMb[:, 0:1], 0.001)
    A.activation(SMb[:, 1:T + 1], ones[:, :], AF.Copy, scale=cc(C_FCH))
    last_e16 = True
    for it, ch in enumerate(plan):
        if ch == "c":
            # const linearization around SM = FC/2 (host-precomputed coeffs)
            A.activation(g1[:, :], INh[:, :], AF.Identity, scale=cc(C_USW0),
                         bias=cc(C_FCH))                            # SMa
            V.tensor_scalar(g2[:, :], g1[:, :], cc(C_FC), None, AL.min)  # SMmid
            V.tensor_scalar(g5[:, :], g2[:, :], cc(C_ILPFC), 1.0,
                            AL.mult, AL.min)                        # q
            V.tensor_tensor(eSoil[:, :], PEh[:, :], g5[:, :], AL.mult)   # e
            V.tensor_tensor(g0[:, :], g2[:, :], eSoil[:, :], AL.subtract)  # fval
            V.tensor_scalar(g3[:, :], g1[:, :], cc(C_FC), None, AL.is_lt)  # mFC
            A.activation(g2[:, :], INh[:, :], AF.Identity, scale=cc(C_NSWP0),
                         bias=1.0)                                  # 1-IN*swp0
            V.tensor_scalar(g7[:, :], g5[:, :], 1.0, None, AL.is_lt)     # mEF
            V.tensor_tensor(g4[:, :], g7[:, :], PETinv16[:, :], AL.mult)
            A.activation(g7[:, :], g4[:, :], AF.Copy, scale=-1.0, bias=1.0)
            V.tensor_tensor(g4[:, :], g3[:, :], g2[:, :], AL.mult)
            V.tensor_tensor(g3[:, :], g4[:, :], g7[:, :], AL.mult)
            V.tensor_scalar(aPrev[:, :], g3[:, :], 0.0, 1.0, AL.max, AL.min)  # a
            A.activation(s0[:, :], g0[:, :], AF.Identity, scale=1.0,
                         bias=cc(C_NFCH))                           # rho
            V.tensor_tensor_scan(s1[:, :], aPrev[:, :], s0[:, :], 0.0,
                                 AL.mult, AL.add)
            V.tensor_scalar(s2[:, :], s1[:, :], cc(C_FCH), NEARZERO,
                            AL.add, AL.max)
            V.tensor_scalar(SMb[:, 1:T + 1], s2[:, :], cc(C_FC), None, AL.min)
            fill(4)
            if "H" in plan:
                # u carry for a following frozen iteration
                V.tensor_scalar(uPrev[:, :], INh[:, :], cc(C_USW0), None,
                                AL.mult)
            last_e16 = True
        elif ch == "g":
            # f16 Newton, simplified derivative a = clip(mFC*(1-IN*swp), 0, 1)
            A.activation(s0[:, :], SMb[:, 0:T], AF.Ln)
            A.activation(g1[:, :], s0[:, :], AF.Exp, scale=cc(C_BETA), bias=cc(C_BLIF))
            A.activation(g2[:, :], s0[:, :], AF.Exp, scale=cc(C_BM1), bias=cc(C_SWPB))
            A.activation(g0[:, :], g1[:, :], AF.Copy, scale=-1.0, bias=1.0)  # 1-sw
            V.tensor_tensor(g6[:, :], g0[:, :], INh[:, :], AL.mult)          # u
            V.tensor_tensor(s3[:, :], SMb[:, 0:T], g6[:, :], AL.add)         # SMa
            V.tensor_scalar(s4[:, :], s3[:, :], cc(C_FC), None, AL.min)      # SMmid
            V.tensor_scalar(g5[:, :], s4[:, :], cc(C_ILPFC), 1.0, AL.mult, AL.min)  # q
            V.tensor_tensor(eSoil[:, :], PEh[:, :], g5[:, :], AL.mult)       # e
            V.tensor_tensor(s6[:, :], s4[:, :], eSoil[:, :], AL.subtract)    # fval
            V.tensor_scalar(g3[:, :], s3[:, :], cc(C_FC), None, AL.is_lt)    # mFC
            V.tensor_tensor(g4[:, :], INh[:, :], g2[:, :], AL.mult)
            A.activation(g2[:, :], g4[:, :], AF.Relu, scale=-1.0, bias=1.0)  # clip(1-IN*swp,0,·)
            V.tensor_tensor(aPrev[:, :], g3[:, :], g2[:, :], AL.mult)        # a
            V.tensor_tensor(s0[:, :], s6[:, :], SMb[:, 1:T + 1], AL.subtract)  # rho
            V.tensor_tensor_scan(s1[:, :], aPrev[:, :], s0[:, :], 0.0,
                                 AL.mult, AL.add)
            V.tensor_tensor(s2[:, :], SMb[:, 1:T + 1], s1[:, :], AL.add)
            V.tensor_scalar(SMb[:, 1:T + 1], s2[:, :], NEARZERO, cc(C_FC),
                            AL.max, AL.min)
            fill(4)
            last_e16 = True
        elif ch == "h":
            A.activation(s0[:, :], SMb[:, 0:T], AF.Ln)
            A.activation(g1[:, :], s0[:, :], AF.Exp, scale=cc(C_BETA), bias=cc(C_BLIF))
            A.activation(g2[:, :], s0[:, :], AF.Exp, scale=cc(C_BM1), bias=cc(C_SWPB))
            A.activation(g0[:, :], g1[:, :], AF.Copy, scale=-1.0, bias=1.0)  # 1-sw
            V.tensor_tensor(uPrev[:, :], g0[:, :], INh[:, :], AL.mult)       # u
            V.tensor_tensor(s3[:, :], SMb[:, 0:T], uPrev[:, :], AL.add)      # SMa
            V.tensor_scalar(s4[:, :], s3[:, :], cc(C_FC), None, AL.min)      # SMmid
            V.tensor_scalar(g5[:, :], s4[:, :], cc(C_ILPFC), 1.0, AL.mult, AL.min)  # q
            V.tensor_tensor(eSoil[:, :], PEh[:, :], g5[:, :], AL.mult)       # e
            V.tensor_tensor(s6[:, :], s4[:, :], eSoil[:, :], AL.subtract)    # fval
            V.tensor_scalar(g3[:, :], s3[:, :], cc(C_FC), None, AL.is_lt)    # mFC
            V.tensor_tensor(g4[:, :], INh[:, :], g2[:, :], AL.mult)
            A.activation(g2[:, :], g4[:, :], AF.Copy, scale=-1.0, bias=1.0)  # 1-IN*swp
            V.tensor_scalar(g7[:, :], g5[:, :], 1.0, None, AL.is_lt)         # mEF
            V.tensor_tensor(g4[:, :], g7[:, :], PETinv16[:, :], AL.mult)
            A.activation(g7[:, :], g4[:, :], AF.Copy, scale=-1.0, bias=1.0)
            V.tensor_tensor(g4[:, :], g3[:, :], g2[:, :], AL.mult)
            V.tensor_tensor(g3[:, :], g4[:, :], g7[:, :], AL.mult)
            V.tensor_scalar(aPrev[:, :], g3[:, :], 0.0, 1.0, AL.max, AL.min)  # a
            V.tensor_tensor(s0[:, :], s6[:, :], SMb[:, 1:T + 1], AL.subtract)  # rho
            V.tensor_tensor_scan(s1[:, :], aPrev[:, :], s0[:, :], 0.0,
                                 AL.mult, AL.add)
            V.tensor_tensor(s2[:, :], SMb[:, 1:T + 1], s1[:, :], AL.add)
            V.tensor_scalar(SMb[:, 1:T + 1], s2[:, :], NEARZERO, cc(C_FC),
                            AL.max, AL.min)
            fill(4)
            last_e16 = True
        elif ch == "H":
            # chord Newton: reuse uPrev/aPrev, recompute residual only
            V.tensor_tensor(s3[:, :], SMb[:, 0:T], uPrev[:, :], AL.add)      # SMa
            V.tensor_scalar(s4[:, :], s3[:, :], cc(C_FC), None, AL.min)      # SMmid
            V.tensor_scalar(g5[:, :], s4[:, :], cc(C_ILPFC), 1.0, AL.mult, AL.min)  # q
            V.tensor_tensor(eSoil[:, :], PEh[:, :], g5[:, :], AL.mult)       # e
            V.tensor_tensor(s6[:, :], s4[:, :], eSoil[:, :], AL.subtract)    # fval
            V.tensor_tensor(s0[:, :], s6[:, :], SMb[:, 1:T + 1], AL.subtract)  # rho
            V.tensor_tensor_scan(s1[:, :], aPrev[:, :], s0[:, :], 0.0,
                                 AL.mult, AL.add)
            V.tensor_tensor(s2[:, :], SMb[:, 1:T + 1], s1[:, :], AL.add)
            V.tensor_scalar(SMb[:, 1:T + 1], s2[:, :], NEARZERO, cc(C_FC),
                            AL.max, AL.min)
            fill(4)
            last_e16 = True
        elif ch == "P":
            A.activation(s0[:, :], SMb[:, 0:T], AF.Ln)
            A.activation(g1[:, :], s0[:, :], AF.Exp, scale=cc(C_BETA), bias=cc(C_BLIF))
            A.activation(g0[:, :], g1[:, :], AF.Copy, scale=-1.0, bias=1.0)  # 1-sw
            V.tensor_tensor(g6[:, :], g0[:, :], INh[:, :], AL.mult)          # u
            V.tensor_tensor(s3[:, :], SMb[:, 0:T], g6[:, :], AL.add)         # SMa
            V.tensor_scalar(s4[:, :], s3[:, :], cc(C_FC), None, AL.min)      # SMmid
            V.tensor_scalar(g5[:, :], s4[:, :], cc(C_ILPFC), 1.0, AL.mult, AL.min)  # q
            V.tensor_tensor(eSoil[:, :], PEh[:, :], g5[:, :], AL.mult)       # e
            V.tensor_tensor(s6[:, :], s4[:, :], eSoil[:, :], AL.subtract)    # fval
            V.tensor_tensor(s0[:, :], s6[:, :], SMb[:, 1:T + 1], AL.subtract)  # rho
            V.tensor_tensor_scan(s1[:, :], aPrev[:, :], s0[:, :], 0.0,
                                 AL.mult, AL.add)
            V.tensor_tensor(s2[:, :], SMb[:, 1:T + 1], s1[:, :], AL.add)
            V.tensor_scalar(SMb[:, 1:T + 1], s2[:, :], NEARZERO, cc(C_FC),
                            AL.max, AL.min)
            fill(4)
            last_e16 = True
        elif ch == "s":
            A.activation(s0[:, :], SMb[:, 0:T], AF.Ln)
            A.activation(s1[:, :], s0[:, :], AF.Exp, scale=cc(C_BETA), bias=cc(C_BLIF))
            A.activation(s2[:, :], s0[:, :], AF.Exp, scale=cc(C_BM1), bias=cc(C_SWPB))
            A.activation(s0[:, :], s1[:, :], AF.Copy, scale=-1.0, bias=1.0)
            V.tensor_tensor(s1[:, :], s0[:, :], INb[:, :], AL.mult)          # u
            V.tensor_tensor(s3[:, :], SMb[:, 0:T], s1[:, :], AL.add)         # SMa
            V.tensor_scalar(s4[:, :], s3[:, :], cc(C_FC), None, AL.min)      # SMmid
            V.tensor_scalar(s5[:, :], s4[:, :], cc(C_ILPFC), 1.0, AL.mult, AL.min)
            V.tensor_tensor(ebF32[:, :], PEp[:, :], s5[:, :], AL.mult)       # e
            V.tensor_tensor(s6[:, :], s4[:, :], ebF32[:, :], AL.subtract)    # fval
            V.tensor_scalar(s0[:, :], s3[:, :], cc(C_FC), None, AL.is_lt)
            V.tensor_tensor(s1[:, :], INb[:, :], s2[:, :], AL.mult)
            A.activation(s1[:, :], s1[:, :], AF.Copy, scale=-1.0, bias=1.0)
            V.tensor_scalar(s2[:, :], s5[:, :], 1.0, None, AL.is_lt)
            V.tensor_tensor(s2[:, :], s2[:, :], PETinv[:, :], AL.mult)
            A.activation(s2[:, :], s2[:, :], AF.Copy, scale=-1.0, bias=1.0)
            V.tensor_tensor(s3[:, :], s0[:, :], s1[:, :], AL.mult)
            V.tensor_tensor(s4[:, :], s3[:, :], s2[:, :], AL.mult)
            V.tensor_scalar(s5[:, :], s4[:, :], 0.0, 1.0, AL.max, AL.min)
            V.tensor_tensor(s0[:, :], s6[:, :], SMb[:, 1:T + 1], AL.subtract)
            V.tensor_tensor_scan(s1[:, :], s5[:, :], s0[:, :], 0.0,
                                 AL.mult, AL.add)
            V.tensor_tensor(s2[:, :], SMb[:, 1:T + 1], s1[:, :], AL.add)
            V.tensor_scalar(SMb[:, 1:T + 1], s2[:, :], NEARZERO, cc(C_FC),
                            AL.max, AL.min)
            fill(4)
            last_e16 = False
        else:
            raise ValueError(f"bad soil plan char {ch}")

    # SUZIN = (INb - e) - (SM[1:] - SM[:T])
    SUZIN = tl("SUZIN")
    esrc = eSoil if last_e16 else ebF32
    V.tensor_tensor(s0[:, :], INb[:, :], esrc[:, :], AL.subtract)
    V.tensor_tensor(s1[:, :], SMb[:, 1:T + 1], SMb[:, 0:T], AL.subtract)
    V.tensor_tensor(SUZIN[:, :], s0[:, :], s1[:, :], AL.subtract)
    fill(3)

    # ---- SUZ regime iteration: f16 with f32 polish tail ----
    SUZb = tl("SUZb", T + 1); SINP = tl("SINP")
    SUZh = tl("SUZh", T + 1, F16)
    SUZINh = tl("SUZINh", T, F16); SINPh = tl("SINPh", T, F16)
    h0 = tl("h0", T, F16); h2 = tl("h2", T, F16); h3 = tl("h3", T, F16)
    h4 = tl("h4", T, F16); h5 = tl("h5", T, F16); h6 = tl("h6", T, F16)
    V.memset(SUZb[:, 0:1], 0.001)
    V.memset(SUZh[:, 0:1], 0.001)
    A.activation(SINP[:, :], SUZIN[:, :], AF.Identity, bias=cc(C_NPCAP))
    A.activation(SUZINh[:, :], SUZIN[:, :], AF.Copy)
    A.activation(SINPh[:, :], SINP[:, :], AF.Copy)
    for it in range(n_s):
        if it < n_s - tail:
            if it == 0:
                A.activation(h0[:, :], SUZINh[:, :], AF.Identity,
                             bias=c001[:, 0:1])
            else:
                V.tensor_tensor(h0[:, :], SUZh[:, 0:T], SUZINh[:, :], AL.add)
            V.tensor_scalar(h3[:, :], h0[:, :], cc(C_PCAP), cc(C_CA),
                            AL.is_gt, AL.mult)
            V.tensor_scalar(h6[:, :], h0[:, :], cc(C_PCUZ), cc(C_CB),
                            AL.is_gt, AL.mult)
            V.tensor_tensor(h4[:, :], h3[:, :], h6[:, :], AL.add)      # alpha
            V.tensor_tensor(h5[:, :], h4[:, :], SINPh[:, :], AL.mult)
            V.tensor_scalar(h2[:, :], h0[:, :], cc(C_PCUZ), cc(C_C3),
                            AL.is_gt, AL.mult)
            V.tensor_tensor(h3[:, :], h5[:, :], h2[:, :], AL.add)      # beta
            V.tensor_tensor_scan(SUZh[:, 1:T + 1], h4[:, :], h3[:, :], 0.001,
                                 AL.mult, AL.add)
            continue
        prevb = SUZh if it == n_s - tail else SUZb
        V.tensor_tensor(s0[:, :], prevb[:, 0:T], SUZIN[:, :], AL.add)  # S1
        V.tensor_scalar(s3[:, :], s0[:, :], cc(C_PCAP), cc(C_CA),
                        AL.is_gt, AL.mult)
        V.tensor_scalar(s4[:, :], s0[:, :], cc(C_PCUZ), cc(C_CB),
                        AL.is_gt, AL.mult)
        V.tensor_tensor(s5[:, :], s3[:, :], s4[:, :], AL.add)          # alpha
        V.tensor_tensor(s6[:, :], s5[:, :], SINP[:, :], AL.mult)
        V.tensor_scalar(s2[:, :], s0[:, :], cc(C_PCUZ), cc(C_C3),
                        AL.is_gt, AL.mult)
        V.tensor_tensor(s3[:, :], s6[:, :], s2[:, :], AL.add)          # beta
        V.tensor_tensor_scan(SUZb[:, 1:T + 1], s5[:, :], s3[:, :], 0.001,
                             AL.mult, AL.add)

    # ---- post-SUZ / SLZ ----
    V.tensor_tensor(s0[:, :], SUZb[:, 0:T], SUZIN[:, :], AL.add)       # S1
    V.tensor_scalar(s5[:, :], s0[:, :], cc(C_PCAP), cc(C_1K2),
                    AL.min, AL.mult)                                   # (1-K2)*PERC
    A.activation(s2[:, :], s0[:, :], AF.Relu, bias=cc(C_NPCAP))        # S1-PERC
    V.tensor_tensor(s3[:, :], s2[:, :], SUZb[:, 1:T + 1], AL.subtract)  # Q01
    A.activation(s4[:, :], ones[:, :], AF.Copy, scale=cc(C_1K2))
    V.tensor_tensor_scan(s6[:, :], s4[:, :], s5[:, :], 0.001, AL.mult, AL.add)  # SLZ
    A.activation(s0[:, :], s6[:, :], AF.Copy, scale=cc(C_KAP))         # Q2

    # ---- routing conv in fp16 (tap products split V/Scalar) ----
    # (PE- and Pool-engine variants measured slower: Pool tensor ops contend
    # for SBUF with the DVE ~2-3x, and any PE activity downclocks the DVE
    # ~20% chip-wide.)
    QbH = tl("QbH", T + LENF - 1, F16)
    yA, yB, wq0, wq1 = g0, g1, g2, g3
    V.memset(QbH[:, 0:LENF - 1], 0.0)
    V.tensor_tensor(QbH[:, LENF - 1:T + LENF - 1], s3[:, :], s0[:, :], AL.add)  # Q
    base = LENF - 1
    V.tensor_scalar(yA[:, :], QbH[:, base:base + T], cc(C_W0), None, AL.mult)
    nxt = []
    acc = [yA, yB]

    def _tap(k):
        def emit():
            wq = (wq0, wq1)[k % 2]
            if k <= tau:
                A.activation(wq[:, :], QbH[:, base - k:base - k + T], AF.Copy,
                             scale=cc(C_W0 + k))
            else:
                V.tensor_scalar(wq[:, :], QbH[:, base - k:base - k + T],
                                cc(C_W0 + k), None, AL.mult)
            V.tensor_tensor(acc[1][:, :], acc[0][:, :], wq[:, :], AL.add)
            acc[0], acc[1] = acc[1], acc[0]
        return emit

    for k in range(1, LENF):
        nxt.append(_tap(k))
    nxt.append(lambda: nc.gpsimd.dma_start(dout[ci], acc[0][:, :]))
    # anything still pending from the previous chunk runs first
    for fn in filler:
        fn()
    return nxt


# ---------------- host orchestration ----------------
_CACHE = {}


def _get_nc(key=None):
    if key is None:
        key = (SLOT_NS, SOIL_PLAN, N_A, SUZ_TAIL, CONV_TAU)
    if key not in _CACHE:
        _CACHE[key] = build_nc(*key)
    return _CACHE[key]


def cell_layout(p, x_phy):
    """Position i (0..G_PAD-1) holds source cell cells[i]; chunk g=i//128 maps
    to core g%8, slot g//8. Hardest cells first so low slots are hard.
    Difficulty = union of two rankings: coarse-sim SUZ residual and the
    persistence heuristic (a cell is hard if either says so)."""
    G = G_FULL
    dsim = difficulty(p, x_phy)
    Pm = x_phy[:, :, 0].mean(axis=0).astype(np.float64)
    PETm = x_phy[:, :, 2].mean(axis=0).astype(np.float64)
    dcrude = (1.0 - p["parK1"]) * (Pm - 0.7 * PETm > p["parPERC"])

    def ranks(d):
        o = np.argsort(-d, kind="stable")
        r = np.empty(G, np.int64)
        r[o] = np.arange(G)
        return r

    runion = np.minimum(ranks(dsim), ranks(dcrude))
    pad = np.arange(G_PAD - G)
    rall = np.concatenate([runion, runion[pad]])
    order = np.argsort(rall, kind="stable")
    cells = np.concatenate([np.arange(G), pad])[order]
    gchunk = np.arange(G_PAD) // P128
    core_of = gchunk % N_CORES
    return cells, core_of


def kernel(x_phy: np.ndarray, parameters: np.ndarray, trace=False):
    x = np.asarray(x_phy, np.float32)
    par_last = np.asarray(parameters)[-1].astype(np.float32)
    Tn, G, _ = x.shape
    assert Tn == T and G == G_FULL
    p = host_params(par_last)
    cells, core_of = cell_layout(p, x)
    p["_forcing_stats"] = (x[:, :, 0].mean(axis=0).astype(np.float64),
                           x[:, :, 2].mean(axis=0).astype(np.float64))
    consts_all = host_consts(p)[cells]
    xg = x[:, cells, :]
    in_maps = []
    per_core = CHUNKS_PER_CORE * P128
    for c in range(N_CORES):
        idx = np.where(core_of == c)[0]
        blk = np.ascontiguousarray(np.moveaxis(xg[:, idx, :], 0, 1))  # [pc, T, 3]
        in_maps.append({
            "pp": np.ascontiguousarray(blk[:, :, 0]).reshape(CHUNKS_PER_CORE, P128, T),
            "tm": np.ascontiguousarray(blk[:, :, 1]).reshape(CHUNKS_PER_CORE, P128, T),
            "pe": np.ascontiguousarray(blk[:, :, 2]).reshape(CHUNKS_PER_CORE, P128, T),
            "cc": np.ascontiguousarray(consts_all[idx]).reshape(CHUNKS_PER_CORE, P128, NCONST),
        })
    nc = _get_nc()
    res = run_bass_kernel_spmd(nc, in_maps, list(range(N_CORES)), trace=trace)
    out = np.empty((T, G), np.float32)
    for c in range(N_CORES):
        idx = np.where(core_of == c)[0]
        ys = res.results[c]["y"].reshape(per_core, T)
        out[:, cells[idx]] = ys.T  # pad duplicates overwrite identically
    if trace:
        return out, res
    return out
